# revision 31
# baseline (speedup 1.0000x reference)
"""Trainium2 Bass kernel for BottleneckAttention (patch attention).

q patches [160, 5120] from z1_hat (non-overlapping 10x4 unfold),
kv patches [5551, 5120] from z2 (overlapping unfold, Hk=91 x Wk=61),
scores = q @ kv.T / 5120, softmax over kv patches, out = attn @ kv,
folded back to [1, 128, 100, 64].

Sharding: contiguous blocks of 12 kv h-rows per core (8 x 12 = 96 >= 91).
Each core owns the 768 flat positions p = h_local*64 + w (w in [0,64);
positions with w >= 61 or h >= 91 are invalid -- their kv rows are zeroed
so they never touch the numerator, and the host subtracts their exactly
recomputed exp contribution from the denominator. Every core computes all
160 q rows; the host combines with an all-gather softmax.

rev H (fp8 + DoubleRow + tile packing):
- All matmul operands are float8 e4m3. The M=128 q-row block uses
  perf_mode=DoubleRow (two 128-deep k-tiles per pass); the 32-row m1
  block uses 3-way (phase 1) / 4-way (phase 2) 32-column tile_position
  packing, which runs the small matmuls concurrently in disjoint PE
  column groups (DoubleRow forbids non-zero dst partitions).
- exp applies bias=ln(64) so the device works with e' = 64*exp(s); the
  PSUM->SBUF copies subtract 64, giving f' = 64*(e^s - 1) in e4m3's
  well-resolved range. Outputs leave as fp8 partials; the host divides
  by 64 and adds the exact colsum term (centered softmax).
- All inputs ride ONE DMA queue in priority order (z slabs 0-1, q
  halves, then kv in three blocks) -- a single queue sustains ~340 GB/s
  where contending queues share ~250. z slabs 2-3 are built on-chip by
  the idle Vector/Scalar engines. Output DMAs use the gpsimd queue.
- PE program: warmup (clock/pipeline ramp) | phase-1 m0 (512/256 col
  groups) | m0 transposes | m1 chains | phase-2 m0 | m1 transposes |
  phase-2 m1 in three col-packed rounds (small round first so the tail
  transfer is full-width). PSUM->SBUF out-copies alternate between the
  Scalar and Vector engines so neither paces the PE.
- The m1 output block leaves in device-native (tile,row) layout via a
  second DRAM tensor; the host reorders it.
"""

import sys

sys.path.insert(0, "/opt/trn_rl_repo")

import math

import numpy as np
import ml_dtypes

import concourse.bass as bass
import concourse.mybir as mybir

DT = mybir.dt
AF = mybir.ActivationFunctionType
DR = mybir.MatmulPerfMode.DoubleRow

# problem geometry (hardcoded from the reference module)
KC, KH, KW = 128, 10, 4
H, W = 100, 64
NH, NW = H // KH, W // KW          # 10, 16
PQ = NH * NW                       # 160 q patches
D = KC * KH * KW                   # 5120
HK, WK = H - KH + 1, W - KW + 1    # 91, 61
NCORES = 8
HPC = 12                           # kv h-rows per core (8*12 = 96 >= 91)
PKC = HPC * W                      # 768 flat positions per core
T = 6                              # 768 / 128 k-chunks for phase 2
G0H, G1H = 8, 4                    # phase-1 h-groups (8+4 = 12)
N0 = G0H * W                       # 448: contiguous stream for h 0..6
N1 = G1H * W                       # 320: contiguous stream for h 7..11
OFF1 = G0H * W                     # 448: flat offset of group 1
ZROWS = 2 * HPC                    # 24 z rows staged per core
ZFL = ZROWS * W                    # 1536 flat z cols
NT = D // 512                      # 10 phase-2 n-tiles
SCALE = 1.0 / D
LN64 = math.log(64.0)

F8 = ml_dtypes.float8_e4m3fn

_CACHE = {}


def _build_nc():
    nc = bass.Bass()
    # z4: the 4 byte-shifted slabs, staged by the host (one DMA)
    z_d = nc.declare_dram_parameter("z4", [KC, KW, ZFL], DT.float8e4, isOutput=False)
    q_d = nc.declare_dram_parameter("qT3", [KC, KH * KW, PQ], DT.float8e4, isOutput=False)
    kv_d = nc.declare_dram_parameter("kvr", [128, T, D], DT.float8e4, isOutput=False)
    out_d = nc.declare_dram_parameter("out", [128, D], DT.float8e4, isOutput=True)
    out2_d = nc.declare_dram_parameter("out2", [128, 3 * 512], DT.float8e4, isOutput=True)
    den_d = nc.declare_dram_parameter("den", [PQ + 96, 1], DT.float32, isOutput=True)

    from contextlib import ExitStack

    ctx = ExitStack()
    with ctx:
        z_sb = ctx.enter_context(nc.sbuf_tensor([KC, KW, ZFL], DT.float8e4))
        q_sb = ctx.enter_context(nc.sbuf_tensor([KC, KH * KW, PQ], DT.float8e4))
        kv_sb = ctx.enter_context(nc.sbuf_tensor([128, T, D], DT.float8e4))
        e_hi = ctx.enter_context(nc.sbuf_tensor([128, PKC], DT.float32))
        e_lo = ctx.enter_context(nc.sbuf_tensor([96, 256], DT.float32))
        eT_sb = ctx.enter_context(nc.sbuf_tensor([128, T, PQ], DT.float8e4))
        o_hi = ctx.enter_context(nc.sbuf_tensor([128, D], DT.float8e4))
        o_lo = ctx.enter_context(nc.sbuf_tensor([128, 3, 512], DT.float8e4))
        iden = ctx.enter_context(nc.sbuf_tensor([128, 128], DT.float32))
        wz = ctx.enter_context(nc.sbuf_tensor([128, 512], DT.bfloat16))
        bias0 = ctx.enter_context(nc.sbuf_tensor([128, 1], DT.float32))
        dh_sb = ctx.enter_context(nc.sbuf_tensor([128, 1], DT.float32))
        dl_sb = ctx.enter_context(nc.sbuf_tensor([96, 1], DT.float32))

        # phase-1 score accumulators: (h-group, q-half)
        ps_s = [
            ctx.enter_context(nc.psum_tensor(f"ps_s{i}", [128, n], DT.float32))
            for i, n in enumerate((N0, N0, N1, 384))
        ]  # order: g0m0, g0m1, g1m0, g1m1
        # transpose staging / phase-2 accumulators (4 distinct banks)
        ps_t = [
            ctx.enter_context(nc.psum_tensor(f"ps_t{i}", [128, 512], DT.float32))
            for i in range(4)
        ]

        s_z1 = ctx.enter_context(nc.semaphore("s_z1"))
        s_z2 = ctx.enter_context(nc.semaphore("s_z2"))
        s_q0 = ctx.enter_context(nc.semaphore("s_q0"))
        s_q1 = ctx.enter_context(nc.semaphore("s_q1"))
        s_kv1 = ctx.enter_context(nc.semaphore("s_kv1"))
        s_kv2 = ctx.enter_context(nc.semaphore("s_kv2"))
        s_kv3 = ctx.enter_context(nc.semaphore("s_kv3"))
        s_p = ctx.enter_context(nc.semaphore("s_p"))
        s_a = ctx.enter_context(nc.semaphore("s_a"))
        s_v = ctx.enter_context(nc.semaphore("s_v"))
        s_d = ctx.enter_context(nc.semaphore("s_d"))
        s_g = ctx.enter_context(nc.semaphore("s_g"))
        s_o = ctx.enter_context(nc.semaphore("s_o"))

        # rev D schedule:
        #   PE:  warmup | g0m0 | g1m0 | tr_m0 | m1 (3-way col-tiled) |
        #        ph2-m0 (DR) | tr_m1 | ph2-m1 (col-tiled rounds)
        #   s_p: 1 g0m0 | 2 g1m0 | 3-8 tr_m0 | 9 m1 | 10-19 ph2m0 |
        #        20-25 tr_m1 | 26-28 ph2m1 rounds
        #   s_a (ACT): 1 exp0 | 2 exp1 | 3-8 trcopy_m0 | 9 exp_m1 |
        #        10-14 ph2m0 even copies | 15-20 trcopy_m1 | 21-23 round copies
        #   s_d (DVE): 1-5 ph2m0 odd copies;  s_v: 1 dh | 2-4 dl

        with nc.Block() as block:

            @block.gpsimd
            def _(g):
                g.memset(wz[:], 0.0).then_inc(s_g, 1)        # 1: warmup tile
                g.memset(iden[:], 0.0)
                g.affine_select(
                    out=iden[:],
                    in_=iden[:],
                    compare_op=mybir.AluOpType.not_equal,
                    fill=1.0,
                    base=0,
                    pattern=[[-1, 128]],
                    channel_multiplier=1,
                ).then_inc(s_g, 1)                            # 2: identity
                g.memset(bias0[:], LN64).then_inc(s_g, 1)     # 3: exp bias
                # out m0 halves behind the ACT/DVE psum->sbuf copies
                g.wait_ge(s_a, 12)
                g.wait_ge(s_d, 2)
                g.dma_start(out_d[:, 0:2560], o_hi[:, 0:2560]).then_inc(s_o, 16)
                g.wait_ge(s_a, 14)
                g.wait_ge(s_d, 5)
                g.dma_start(out_d[:, 2560:], o_hi[:, 2560:]).then_inc(s_o, 16)
                # q-rows 128..159 output in device-native (c,row) layout;
                # one DMA per phase-2-m1 round, right after its copy
                g.wait_ge(s_a, 18)
                g.dma_start(out2_d[0:64, 0:512], o_lo[0:64, 0, :]).then_inc(s_o, 16)
                g.wait_ge(s_d, 9)
                g.dma_start(out2_d[:, 512:1024], o_lo[:, 1, :]).then_inc(s_o, 16)
                g.wait_ge(s_a, 19)
                g.dma_start(out2_d[:, 1024:1536], o_lo[:, 2, :]).then_inc(s_o, 16)

            @block.sync
            def _(sync):
                # ALL inputs on one queue in priority order: a single queue
                # sustains ~340 GB/s while two contending queues drop to ~250
                sync.dma_start(z_sb[:, 0:2, :], z_d[:, 0:2, :]).then_inc(s_z1, 16)
                sync.dma_start(q_sb[:, 0:20, :], q_d[:, 0:20, :]).then_inc(s_q0, 16)
                sync.dma_start(q_sb[:, 20:40, :], q_d[:, 20:40, :]).then_inc(
                    s_q1, 16
                )
                sync.dma_start(kv_sb[:, :, 0:1536], kv_d[:, :, 0:1536]).then_inc(
                    s_kv1, 16
                )
                sync.dma_start(
                    kv_sb[:, :, 1536:3584], kv_d[:, :, 1536:3584]
                ).then_inc(s_kv2, 16)
                sync.dma_start(
                    kv_sb[:, :, 3584:5120], kv_d[:, :, 3584:5120]
                ).then_inc(s_kv3, 16)
                sync.wait_ge(s_v, 1)
                sync.dma_start(den_d[0:128, :], dh_sb[:]).then_inc(s_o, 16)
                sync.wait_ge(s_v, 4)
                sync.dma_start(den_d[128:224, :], dl_sb[:]).then_inc(s_o, 16)
                sync.wait_ge(s_o, 112)

            @block.tensor
            def _(pe):
                # HAM warmup on the zeroed bf16 tile while input DMAs land.
                pe.wait_ge(s_g, 1)
                for _w in range(5):
                    nc.tensor.matmul(
                        ps_t[0][0:128, 0:512],
                        wz[:, 0:128],
                        wz[:, 0:512],
                        start=(_w == 0),
                        stop=(_w == 4),
                    )
                for _w in range(5):
                    nc.tensor.matmul(
                        ps_t[1][0:128, 0:256],
                        wz[:, 0:128],
                        wz[:, 0:256],
                        start=(_w == 0),
                        stop=(_w == 4),
                    )
                # phase 1 m0: scores[pq0:128, pos] -- DR over (j, j+1) pairs.
                # Step order: all jp=0 pairs (slabs 0-1, q half 0) first, then
                # jp=2 (slabs 2-3, q half 1). Junk cols corrected on host.
                def chain(grp, first):
                    ps = ps_s[grp * 2]
                    for s in range(20):
                        if first and s == 0:
                            pe.wait_ge(s_z1, 16)
                            pe.wait_ge(s_q0, 16)
                        if first and s == 5:
                            pe.wait_ge(s_z2, 2)
                        if first and s == 10:
                            pe.wait_ge(s_q1, 16)
                        blk = s // 5
                        i_, jp = (blk // 2) * 5 + s % 5, (blk % 2) * 2
                        ij = i_ * KW + jp
                        st, sp = s == 0, s == 19
                        base = i_ * W + (OFF1 if grp == 1 else 0)
                        n = N1 if grp == 1 else N0
                        mm = nc.tensor.matmul(
                            ps[:, :],
                            q_sb[:, ij : ij + 2, 0:128],
                            z_sb[:, jp : jp + 2, base : base + n],
                            start=st,
                            stop=sp,
                            perf_mode=DR,
                        )
                    return mm

                chain(0, True).then_inc(s_p, 1)   # s_p = 1
                chain(1, False).then_inc(s_p, 1)  # s_p = 2
                # m0 transposes (fp32): bank k%4; banks 0/1 reused at k=4,5
                pe.wait_ge(s_g, 2)
                TRM0_FREED = [1, 1, 1, 1, 2, 2]
                TRM0_BANKS = [ps_t[0], ps_t[1], ps_t[2], ps_t[3], ps_s[0], ps_s[2]]
                for k in range(6):
                    pe.wait_ge(s_a, TRM0_FREED[k])
                    nc.tensor.matmul(
                        TRM0_BANKS[k][0:128, 0:128],
                        e_hi[:, k * 128 : (k + 1) * 128],
                        iden[0:128, 0:128],
                        is_transpose=True,
                        start=True,
                        stop=True,
                    ).then_inc(s_p, 1)               # s_p = 3..8
                # m1 score chains (q rows 128:160): 3-way col tiling, each
                # tile owns 256 positions; plain fp8 (no DR -- col tiling and
                # DR are mutually exclusive)
                for s in range(40):
                    i_, jp = s % 10, (s // 10) % 2 * 2
                    j_ = jp + (0 if s < 20 else 1)
                    ij = i_ * KW + j_
                    st, sp_ = s == 0 or s == 1, s == 38 or s == 39
                    for c in range(3):
                        mm = nc.tensor.matmul(
                            ps_s[1][32 * c : 32 * c + 32, 0:256],
                            q_sb[:, ij, 128:160],
                            z_sb[:, j_, i_ * W + 256 * c : i_ * W + 256 * c + 256],
                            start=(s == 0),
                            stop=(s == 39),
                            tile_position=(0, 32 * c),
                        )
                mm.then_inc(s_p, 1)  # s_p = 9
                # phase 2 m0 (DR over k-chunk pairs)
                pe.wait_ge(s_a, 8)
                PH2_BANK_FREE = [(0, 0), (0, 0), (0, 0), (0, 0),
                                 (1, 10), (2, 1), (1, 11), (2, 2),
                                 (1, 12), (2, 3)]
                for gidx in range(NT):
                    if gidx == 0:
                        pe.wait_ge(s_kv1, 16)
                    elif gidx == 3:
                        pe.wait_ge(s_kv2, 16)
                    elif gidx == 7:
                        pe.wait_ge(s_kv3, 16)
                    kind, val = PH2_BANK_FREE[gidx]
                    if kind == 1:
                        pe.wait_ge(s_a, val)
                    elif kind == 2:
                        pe.wait_ge(s_d, val)
                    for tp in range(T // 2):
                        t = 2 * tp
                        mm = nc.tensor.matmul(
                            ps_t[gidx % 4][0:128, 0:512],
                            eT_sb[:, t : t + 2, 0:128],
                            kv_sb[:, t : t + 2, gidx * 512 : (gidx + 1) * 512],
                            start=(tp == 0),
                            stop=(tp == T // 2 - 1),
                            perf_mode=DR,
                        )
                    mm.then_inc(s_p, 1)  # s_p = 10+gidx
                # m1 transposes from e_lo [96, 256]: chunk t lives at
                # partitions 32*(t//2), cols 128*(t%2)
                TRM1_A = [9, 9, 9, 9, 15, 9]
                TRM1_D = [0, 0, 0, 0, 0, 6]
                TRM1_BANKS = [ps_s[3], ps_s[0], ps_s[2], ps_s[1], ps_s[3], ps_s[0]]
                for k in range(6):
                    pe.wait_ge(s_a, TRM1_A[k])
                    if TRM1_D[k]:
                        pe.wait_ge(s_d, TRM1_D[k])
                    pb = 32 * (k // 2)
                    cb = 128 * (k % 2)
                    nc.tensor.matmul(
                        TRM1_BANKS[k][0:128, 0:32],
                        e_lo[pb : pb + 32, cb : cb + 128],
                        iden[pb : pb + 32, pb : pb + 32],
                        is_transpose=True,
                        start=True,
                        stop=True,
                    ).then_inc(s_p, 1)  # s_p = 20..25
                # phase 2 m1: rounds of col-tiled n-tiles sharing one psum
                # bank at partition bases 0/32/64/96; the small (2-tile)
                # round g8-9 runs FIRST so the last round's copy+DMA tail is
                # a full-width transfer
                pe.wait_ge(s_a, 17)
                pe.wait_ge(s_d, 8)
                RBANKS = [ps_s[0], ps_t[2], ps_t[3]]
                RG = [(8, 2), (0, 4), (4, 4)]
                for r in range(3):
                    g0, ntile = RG[r]
                    bank = RBANKS[r]
                    for t in range(T):
                        for c in range(ntile):
                            gcol = (g0 + c) * 512
                            mm = nc.tensor.matmul(
                                bank[32 * c : 32 * c + 32, 0:512],
                                eT_sb[:, t, 128:160],
                                kv_sb[:, t, gcol : gcol + 512],
                                start=(t == 0),
                                stop=(t == T - 1),
                                tile_position=(0, 32 * c),
                            )
                    mm.then_inc(s_p, 1)  # s_p = 26+r

            @block.scalar
            def _(act):
                # preload the EXP activation table while idle
                act.wait_ge(s_g, 3)
                nc.scalar.activation(
                    e_hi[0:1, 0:1], bias0[0:1, 0:1], AF.Exp, bias=0.0, scale=1.0
                )
                # build z slab 3 on-chip (slab0 shifted by 3 bytes)
                act.wait_ge(s_z1, 16)
                nc.scalar.activation(
                    z_sb[:, 3, 0 : ZFL - 3], z_sb[:, 0, 3:ZFL], AF.Copy
                ).then_inc(s_z2, 1)

                def expcall(ps, esl, b):
                    # e' = 64 * exp(s): bias ln(64) keeps the centered
                    # f' = e' - 64 in e4m3's well-resolved range.
                    nc.scalar.activation(
                        esl, ps, AF.Exp, bias=b, scale=SCALE
                    ).then_inc(s_a, 1)

                act.wait_ge(s_p, 1)
                expcall(ps_s[0][:, :], e_hi[:, 0:N0], bias0[:, :])        # s_a=1
                act.wait_ge(s_p, 2)
                expcall(ps_s[2][:, :], e_hi[:, OFF1 : OFF1 + N1], bias0[:, :])  # 2
                TRC0_BANKS = [ps_t[0], ps_t[1], ps_t[2], ps_t[3], ps_s[0], ps_s[2]]
                for k in range(6):
                    # transposed m0 chunks -> f' = e' - 64 (cast to e4m3)
                    act.wait_ge(s_p, 3 + k)
                    nc.scalar.activation(
                        eT_sb[:, k, 0:128],
                        TRC0_BANKS[k][0:128, 0:128],
                        AF.Copy,
                        bias=-64.0,
                    ).then_inc(s_a, 1)                                   # s_a=3..8
                act.wait_ge(s_p, 9)
                expcall(ps_s[1][0:96, 0:256], e_lo[0:96, 0:256], bias0[0:96, :])  # 9
                for h in range(5):
                    # ph2-m0 even-g out copies (odd g on the DVE)
                    g2 = 2 * h
                    act.wait_ge(s_p, 10 + g2)
                    nc.scalar.activation(
                        o_hi[:, g2 * 512 : (g2 + 1) * 512],
                        ps_t[g2 % 4][0:128, 0:512],
                        AF.Copy,
                    ).then_inc(s_a, 1)  # s_a = 10+h
                for k, tb in ((0, 3), (2, 2), (4, 3)):
                    act.wait_ge(s_p, 20 + k)
                    nc.scalar.activation(
                        eT_sb[:, k, 128:160],
                        ps_s[tb][0:128, 0:32],
                        AF.Copy,
                        bias=-64.0,
                    ).then_inc(s_a, 1)  # s_a = 15, 16, 17
                act.wait_ge(s_p, 26)
                nc.scalar.activation(
                    o_lo[0:64, 0, :], ps_s[0][0:64, 0:512], AF.Copy
                ).then_inc(s_a, 1)  # s_a = 18: round 0 (g8-9)
                act.wait_ge(s_p, 28)
                nc.scalar.activation(
                    o_lo[:, 2, :], ps_t[3][0:128, 0:512], AF.Copy
                ).then_inc(s_a, 1)  # s_a = 19: round 2 (g4-7)

            @block.vector
            def _(dve):
                # build z slab 2 on-chip (slab0 shifted by 2 bytes)
                dve.wait_ge(s_z1, 16)
                nc.vector.tensor_copy(
                    z_sb[:, 2, 0 : ZFL - 2], z_sb[:, 0, 2:ZFL]
                ).then_inc(s_z2, 1)
                dve.wait_ge(s_a, 2)
                nc.vector.reduce_sum(
                    dh_sb[:], e_hi[:, :], axis=mybir.AxisListType.X
                ).then_inc(s_v, 1)
                # ph2-m0 odd-g out copies
                for h in range(5):
                    g2 = 2 * h + 1
                    dve.wait_ge(s_p, 10 + g2)
                    nc.vector.tensor_copy(
                        o_hi[:, g2 * 512 : (g2 + 1) * 512],
                        ps_t[g2 % 4][0:128, 0:512],
                    ).then_inc(s_d, 1)  # s_d = 1+h
                for k, tb in ((1, 0), (3, 1), (5, 0)):
                    dve.wait_ge(s_p, 20 + k)
                    nc.vector.tensor_scalar_add(
                        eT_sb[:, k, 128:160],
                        ps_s[tb][0:128, 0:32],
                        -64.0,
                    ).then_inc(s_d, 1)  # s_d = 6, 7, 8
                dve.wait_ge(s_a, 9)
                for c in range(3):
                    nc.vector.reduce_sum(
                        dl_sb[32 * c : 32 * c + 32, :],
                        e_lo[32 * c : 32 * c + 32, :],
                        axis=mybir.AxisListType.X,
                    ).then_inc(s_v, 1)  # s_v = 2..4
                dve.wait_ge(s_p, 27)
                nc.vector.tensor_copy(
                    o_lo[:, 1, :], ps_t[2][0:128, 0:512]
                ).then_inc(s_d, 1)  # s_d = 9: ph2-m1 round 1 (g0-3)

    return nc


def _host_prep(z1_hat, z2):
    z1 = np.asarray(z1_hat, dtype=np.float32)[0]  # [128, 100, 64]
    z2a = np.asarray(z2, dtype=np.float32)[0]

    # q patches [160, 5120] and lhsT layout qT3 [128, 40, 160]
    q = z1.reshape(KC, NH, KH, NW, KW).transpose(1, 3, 0, 2, 4).reshape(PQ, D)
    qT3 = np.ascontiguousarray(q.reshape(PQ, KC, KH * KW).transpose(1, 2, 0).astype(F8))

    # padded z2: rows 100..111 zero
    z_pad = np.zeros((KC, 112, W), dtype=np.float32)
    z_pad[:, :H] = z2a

    # sliding kv patches from padded z2
    sw = np.lib.stride_tricks.sliding_window_view(z_pad, (KH, KW), axis=(1, 2))
    # sw: [128, 103, 61, 10, 4]; patch(h, w) = sw[:, h, w]

    q64 = q.astype(np.float64)
    ij_off = (np.arange(KH)[:, None] * W + np.arange(KW)[None, :]).reshape(-1)  # [40]

    in_maps = []
    corrs = []
    for core in range(NCORES):
        h0 = HPC * core
        zf = z_pad[:, h0 : h0 + ZROWS, :].reshape(KC, ZFL)
        # the 4 byte-shifted slabs, quantized once so all shifts agree
        zf8 = np.zeros((KC, ZFL + KW), dtype=F8)
        zf8[:, :ZFL] = zf.astype(F8)
        z4 = np.stack([zf8[:, s : s + ZFL] for s in range(KW)], axis=1)
        # kv rows indexed by flat position p = h_local*64 + w
        kvp = np.zeros((PKC, D), dtype=np.float32)
        hh = np.arange(PKC) // W
        ww = np.arange(PKC) % W
        real = (ww < WK) & (h0 + hh < HK)
        ridx = np.nonzero(real)[0]
        kvp[ridx] = (
            sw[:, h0 + hh[ridx], ww[ridx]].transpose(1, 0, 2, 3).reshape(-1, D)
        )
        kvr = np.ascontiguousarray(
            kvp.reshape(T, 128, D).transpose(1, 0, 2).astype(F8)
        )
        in_maps.append(
            {
                "z4": np.ascontiguousarray(z4),
                "qT3": qT3,
                "kvr": kvr,
            }
        )
        # denominator correction: computed-but-invalid columns (w >= 61 or
        # h >= 91) polluted the on-chip row-sum; subtract their exactly
        # recomputed exp contribution.
        bad = np.nonzero(~real)[0]
        win = zf.astype(np.float64)[:, bad[:, None] + ij_off[None, :]]  # [128,nb,40]
        patches = win.transpose(1, 0, 2).reshape(len(bad), D)  # d-order (c, i, j)
        s_bad = q64 @ patches.T  # [160, nb]
        corrs.append(np.exp(s_bad * SCALE).sum(axis=1))

    corr = np.sum(corrs, axis=0)
    # centered softmax: device returns f' @ kv with f' = 64*(e-1); host adds
    # the exact colsum term sum_k kv[k, :] over all real patches (all cores).
    swr = sw[:, :HK, :WK]
    colsum = swr.astype(np.float64).sum(axis=(1, 2)).reshape(D)  # [5120]
    return in_maps, corr, colsum


def kernel(z1_hat, z2):
    from concourse.bass_utils import run_bass_kernel_spmd

    in_maps, corr, colsum = _host_prep(z1_hat, z2)
    if "nc" not in _CACHE:
        _CACHE["nc"] = _build_nc()
    nc = _CACHE["nc"]
    res = run_bass_kernel_spmd(nc, in_maps, list(range(NCORES)))
    num = np.broadcast_to(colsum, (PQ, D)).astype(np.float64).copy()
    den = -corr
    for r in res.results:
        num[0:128] += r["out"].astype(np.float64) / 64.0
        o2 = r["out2"].astype(np.float64).reshape(4, 32, 3, 512)
        m1 = np.concatenate(
            [
                o2[:, :, 1, :].transpose(1, 0, 2).reshape(32, 2048),
                o2[:, :, 2, :].transpose(1, 0, 2).reshape(32, 2048),
                o2[0:2, :, 0, :].transpose(1, 0, 2).reshape(32, 1024),
            ],
            axis=1,
        )
        num[128:160] += m1 / 64.0
        dv = r["den"].astype(np.float64)[:, 0] / 64.0
        den = den + np.concatenate(
            [dv[0:128], dv[128:160] + dv[160:192] + dv[192:224]]
        )
    out = (num / den[:, None]).astype(np.float32)
    # fold patches back: [160, 5120] -> [1, 128, 100, 64]
    out = out.reshape(NH, NW, KC, KH, KW).transpose(2, 0, 3, 1, 4)
    return np.ascontiguousarray(out.reshape(1, KC, H, W))


# revision 33
# speedup vs baseline: 1.0197x; 1.0197x over previous
"""Trainium2 Bass kernel for BottleneckAttention (patch attention).

q patches [160, 5120] from z1_hat (non-overlapping 10x4 unfold),
kv patches [5551, 5120] from z2 (overlapping unfold, Hk=91 x Wk=61),
scores = q @ kv.T / 5120, softmax over kv patches, out = attn @ kv,
folded back to [1, 128, 100, 64].

Sharding: contiguous blocks of 12 kv h-rows per core (8 x 12 = 96 >= 91).
Each core owns the 768 flat positions p = h_local*64 + w (w in [0,64);
positions with w >= 61 or h >= 91 are invalid -- their kv rows are zeroed
so they never touch the numerator, and the host subtracts their exactly
recomputed exp contribution from the denominator. Every core computes all
160 q rows; the host combines with an all-gather softmax.

rev H (fp8 + DoubleRow + tile packing):
- All matmul operands are float8 e4m3. The M=128 q-row block uses
  perf_mode=DoubleRow (two 128-deep k-tiles per pass); the 32-row m1
  block uses 3-way (phase 1) / 4-way (phase 2) 32-column tile_position
  packing, which runs the small matmuls concurrently in disjoint PE
  column groups (DoubleRow forbids non-zero dst partitions).
- exp applies bias=ln(64) so the device works with e' = 64*exp(s); the
  PSUM->SBUF copies subtract 64, giving f' = 64*(e^s - 1) in e4m3's
  well-resolved range. Outputs leave as fp8 partials; the host divides
  by 64 and adds the exact colsum term (centered softmax).
- All inputs ride ONE DMA queue in priority order (z slabs 0-1, q
  halves, then kv in three blocks) -- a single queue sustains ~340 GB/s
  where contending queues share ~250. z slabs 2-3 are built on-chip by
  the idle Vector/Scalar engines. Output DMAs use the gpsimd queue.
- PE program: warmup (clock/pipeline ramp) | phase-1 m0 (512/256 col
  groups) | m0 transposes | m1 chains | phase-2 m0 | m1 transposes |
  phase-2 m1 in three col-packed rounds (small round first so the tail
  transfer is full-width). PSUM->SBUF out-copies alternate between the
  Scalar and Vector engines so neither paces the PE.
- The m1 output block leaves in device-native (tile,row) layout via a
  second DRAM tensor; the host reorders it.
"""

import sys

sys.path.insert(0, "/opt/trn_rl_repo")

import math

import numpy as np
import ml_dtypes

import concourse.bass as bass
import concourse.mybir as mybir

DT = mybir.dt
AF = mybir.ActivationFunctionType
DR = mybir.MatmulPerfMode.DoubleRow

# problem geometry (hardcoded from the reference module)
KC, KH, KW = 128, 10, 4
H, W = 100, 64
NH, NW = H // KH, W // KW          # 10, 16
PQ = NH * NW                       # 160 q patches
D = KC * KH * KW                   # 5120
HK, WK = H - KH + 1, W - KW + 1    # 91, 61
NCORES = 8
HPC = 12                           # kv h-rows per core (8*12 = 96 >= 91)
PKC = HPC * W                      # 768 flat positions per core
T = 6                              # 768 / 128 k-chunks for phase 2
G0H, G1H = 8, 4                    # phase-1 h-groups (8+4 = 12)
N0 = G0H * W                       # 448: contiguous stream for h 0..6
N1 = G1H * W                       # 320: contiguous stream for h 7..11
OFF1 = G0H * W                     # 448: flat offset of group 1
ZROWS = 2 * HPC                    # 24 z rows staged per core
ZFL = ZROWS * W                    # 1536 flat z cols
NT = D // 512                      # 10 phase-2 n-tiles
SCALE = 1.0 / D
LN64 = math.log(64.0)

F8 = ml_dtypes.float8_e4m3fn

_CACHE = {}


def _build_nc():
    nc = bass.Bass()
    # z4: the 4 byte-shifted slabs, staged by the host (one DMA)
    z_d = nc.declare_dram_parameter("z4", [KC, KW, ZFL], DT.float8e4, isOutput=False)
    q_d = nc.declare_dram_parameter("qT3", [KC, KH * KW, PQ], DT.float8e4, isOutput=False)
    kv_d = nc.declare_dram_parameter("kvr", [128, T, D], DT.float8e4, isOutput=False)
    out_d = nc.declare_dram_parameter("out", [128, D], DT.float8e4, isOutput=True)
    out2_d = nc.declare_dram_parameter("out2", [128, 3 * 512], DT.float8e4, isOutput=True)
    den_d = nc.declare_dram_parameter("den", [PQ + 96, 1], DT.float32, isOutput=True)

    from contextlib import ExitStack

    ctx = ExitStack()
    with ctx:
        z_sb = ctx.enter_context(nc.sbuf_tensor([KC, KW, ZFL], DT.float8e4))
        q_sb = ctx.enter_context(nc.sbuf_tensor([KC, KH * KW, PQ], DT.float8e4))
        kv_sb = ctx.enter_context(nc.sbuf_tensor([128, T, D], DT.float8e4))
        e_hi = ctx.enter_context(nc.sbuf_tensor([128, PKC], DT.float32))
        e_lo = ctx.enter_context(nc.sbuf_tensor([96, 256], DT.float32))
        eT_sb = ctx.enter_context(nc.sbuf_tensor([128, T, PQ], DT.float8e4))
        o_hi = ctx.enter_context(nc.sbuf_tensor([128, D], DT.float8e4))
        o_lo = ctx.enter_context(nc.sbuf_tensor([128, 3, 512], DT.float8e4))
        iden = ctx.enter_context(nc.sbuf_tensor([128, 128], DT.float32))
        wz = ctx.enter_context(nc.sbuf_tensor([128, 512], DT.bfloat16))
        bias0 = ctx.enter_context(nc.sbuf_tensor([128, 1], DT.float32))
        dh_sb = ctx.enter_context(nc.sbuf_tensor([128, 1], DT.float32))
        dl_sb = ctx.enter_context(nc.sbuf_tensor([96, 1], DT.float32))

        # phase-1 score accumulators: (h-group, q-half)
        ps_s = [
            ctx.enter_context(nc.psum_tensor(f"ps_s{i}", [128, n], DT.float32))
            for i, n in enumerate((N0, N0, N1, 384))
        ]  # order: g0m0, g0m1, g1m0, g1m1
        # transpose staging / phase-2 accumulators (4 distinct banks)
        ps_t = [
            ctx.enter_context(nc.psum_tensor(f"ps_t{i}", [128, 512], DT.float32))
            for i in range(4)
        ]

        s_z1 = ctx.enter_context(nc.semaphore("s_z1"))
        s_z2 = ctx.enter_context(nc.semaphore("s_z2"))
        s_q0 = ctx.enter_context(nc.semaphore("s_q0"))
        s_q1 = ctx.enter_context(nc.semaphore("s_q1"))
        s_kv1 = ctx.enter_context(nc.semaphore("s_kv1"))
        s_kv2 = ctx.enter_context(nc.semaphore("s_kv2"))
        s_kv3 = ctx.enter_context(nc.semaphore("s_kv3"))
        s_p = ctx.enter_context(nc.semaphore("s_p"))
        s_a = ctx.enter_context(nc.semaphore("s_a"))
        s_v = ctx.enter_context(nc.semaphore("s_v"))
        s_d = ctx.enter_context(nc.semaphore("s_d"))
        s_g = ctx.enter_context(nc.semaphore("s_g"))
        s_o = ctx.enter_context(nc.semaphore("s_o"))

        # rev D schedule:
        #   PE:  warmup | g0m0 | g1m0 | tr_m0 | m1 (3-way col-tiled) |
        #        ph2-m0 (DR) | tr_m1 | ph2-m1 (col-tiled rounds)
        #   s_p: 1 g0m0 | 2 g1m0 | 3-8 tr_m0 | 9 m1 | 10-19 ph2m0 |
        #        20-25 tr_m1 | 26-28 ph2m1 rounds
        #   s_a (ACT): 1 exp0 | 2 exp1 | 3-8 trcopy_m0 | 9 exp_m1 |
        #        10-14 ph2m0 even copies | 15-20 trcopy_m1 | 21-23 round copies
        #   s_d (DVE): 1-5 ph2m0 odd copies;  s_v: 1 dh | 2-4 dl

        with nc.Block() as block:

            @block.gpsimd
            def _(g):
                g.memset(wz[:], 0.0).then_inc(s_g, 1)        # 1: warmup tile
                g.memset(iden[:], 0.0)
                g.affine_select(
                    out=iden[:],
                    in_=iden[:],
                    compare_op=mybir.AluOpType.not_equal,
                    fill=1.0,
                    base=0,
                    pattern=[[-1, 128]],
                    channel_multiplier=1,
                ).then_inc(s_g, 1)                            # 2: identity
                g.memset(bias0[:], LN64).then_inc(s_g, 1)     # 3: exp bias
                # out m0 halves behind the ACT/DVE psum->sbuf copies
                g.wait_ge(s_a, 12)
                g.wait_ge(s_d, 2)
                g.dma_start(out_d[:, 0:2560], o_hi[:, 0:2560]).then_inc(s_o, 16)
                g.wait_ge(s_a, 14)
                g.wait_ge(s_d, 5)
                g.dma_start(out_d[:, 2560:], o_hi[:, 2560:]).then_inc(s_o, 16)
                # q-rows 128..159 output in device-native (c,row) layout;
                # one DMA per phase-2-m1 round, right after its copy
                g.wait_ge(s_a, 18)
                g.dma_start(out2_d[0:64, 0:512], o_lo[0:64, 0, :]).then_inc(s_o, 16)
                g.wait_ge(s_d, 9)
                g.dma_start(out2_d[:, 512:1024], o_lo[:, 1, :]).then_inc(s_o, 16)
                g.wait_ge(s_a, 19)
                g.dma_start(out2_d[:, 1024:1536], o_lo[:, 2, :]).then_inc(s_o, 16)

            @block.sync
            def _(sync):
                # ALL inputs on one queue in priority order: a single queue
                # sustains ~340 GB/s while two contending queues drop to ~250
                sync.dma_start(z_sb[:, 0:2, :], z_d[:, 0:2, :]).then_inc(s_z1, 16)
                sync.dma_start(q_sb[:, 0:20, :], q_d[:, 0:20, :]).then_inc(s_q0, 16)
                sync.dma_start(q_sb[:, 20:40, :], q_d[:, 20:40, :]).then_inc(
                    s_q1, 16
                )
                sync.dma_start(kv_sb[:, :, 0:1536], kv_d[:, :, 0:1536]).then_inc(
                    s_kv1, 16
                )
                sync.dma_start(
                    kv_sb[:, :, 1536:3584], kv_d[:, :, 1536:3584]
                ).then_inc(s_kv2, 16)
                sync.dma_start(
                    kv_sb[:, :, 3584:5120], kv_d[:, :, 3584:5120]
                ).then_inc(s_kv3, 16)
                sync.wait_ge(s_v, 1)
                sync.dma_start(den_d[0:128, :], dh_sb[:]).then_inc(s_o, 16)
                sync.wait_ge(s_v, 4)
                sync.dma_start(den_d[128:224, :], dl_sb[:]).then_inc(s_o, 16)
                sync.wait_ge(s_o, 112)

            @block.tensor
            def _(pe):
                # HAM warmup on the zeroed bf16 tile while input DMAs land.
                pe.wait_ge(s_g, 1)
                for _w in range(5):
                    nc.tensor.matmul(
                        ps_t[0][0:128, 0:512],
                        wz[:, 0:128],
                        wz[:, 0:512],
                        start=(_w == 0),
                        stop=(_w == 4),
                    )
                for _w in range(5):
                    nc.tensor.matmul(
                        ps_t[1][0:128, 0:256],
                        wz[:, 0:128],
                        wz[:, 0:256],
                        start=(_w == 0),
                        stop=(_w == 4),
                    )
                # phase 1 m0: scores[pq0:128, pos] -- DR over (j, j+1) pairs.
                # Step order: all jp=0 pairs (slabs 0-1, q half 0) first, then
                # jp=2 (slabs 2-3, q half 1). Junk cols corrected on host.
                def chain(grp, first):
                    ps = ps_s[grp * 2]
                    for s in range(20):
                        if first and s == 0:
                            pe.wait_ge(s_z1, 16)
                            pe.wait_ge(s_q0, 16)
                        if first and s == 5:
                            pe.wait_ge(s_z2, 2)
                        if first and s == 10:
                            pe.wait_ge(s_q1, 16)
                        blk = s // 5
                        i_, jp = (blk // 2) * 5 + s % 5, (blk % 2) * 2
                        ij = i_ * KW + jp
                        st, sp = s == 0, s == 19
                        base = i_ * W + (OFF1 if grp == 1 else 0)
                        n = N1 if grp == 1 else N0
                        mm = nc.tensor.matmul(
                            ps[:, :],
                            q_sb[:, ij : ij + 2, 0:128],
                            z_sb[:, jp : jp + 2, base : base + n],
                            start=st,
                            stop=sp,
                            perf_mode=DR,
                        )
                    return mm

                chain(0, True).then_inc(s_p, 1)   # s_p = 1
                chain(1, False).then_inc(s_p, 1)  # s_p = 2
                # m0 transposes (fp32): bank k%4; banks 0/1 reused at k=4,5
                pe.wait_ge(s_g, 2)
                TRM0_FREED = [1, 1, 1, 1, 2, 2]
                TRM0_BANKS = [ps_t[0], ps_t[1], ps_t[2], ps_t[3], ps_s[0], ps_s[2]]
                for k in range(6):
                    pe.wait_ge(s_a, TRM0_FREED[k])
                    nc.tensor.matmul(
                        TRM0_BANKS[k][0:128, 0:128],
                        e_hi[:, k * 128 : (k + 1) * 128],
                        iden[0:128, 0:128],
                        is_transpose=True,
                        start=True,
                        stop=True,
                    ).then_inc(s_p, 1)               # s_p = 3..8
                # m1 score chains (q rows 128:160): 3-way col tiling, each
                # tile owns 256 positions; plain fp8 (no DR -- col tiling and
                # DR are mutually exclusive)
                for s in range(40):
                    i_, jp = s % 10, (s // 10) % 2 * 2
                    j_ = jp + (0 if s < 20 else 1)
                    ij = i_ * KW + j_
                    st, sp_ = s == 0 or s == 1, s == 38 or s == 39
                    for c in range(3):
                        mm = nc.tensor.matmul(
                            ps_s[1][32 * c : 32 * c + 32, 0:256],
                            q_sb[:, ij, 128:160],
                            z_sb[:, j_, i_ * W + 256 * c : i_ * W + 256 * c + 256],
                            start=(s == 0),
                            stop=(s == 39),
                            tile_position=(0, 32 * c),
                        )
                mm.then_inc(s_p, 1)  # s_p = 9
                # phase 2 m0 (DR over k-chunk pairs)
                pe.wait_ge(s_a, 8)
                PH2_BANK_FREE = [(0, 0), (0, 0), (0, 0), (0, 0),
                                 (1, 10), (2, 1), (1, 11), (2, 2),
                                 (1, 12), (2, 3)]
                for gidx in range(NT):
                    if gidx == 0:
                        pe.wait_ge(s_kv1, 16)
                    elif gidx == 3:
                        pe.wait_ge(s_kv2, 16)
                    elif gidx == 7:
                        pe.wait_ge(s_kv3, 16)
                    kind, val = PH2_BANK_FREE[gidx]
                    if kind == 1:
                        pe.wait_ge(s_a, val)
                    elif kind == 2:
                        pe.wait_ge(s_d, val)
                    for tp in range(T // 2):
                        t = 2 * tp
                        mm = nc.tensor.matmul(
                            ps_t[gidx % 4][0:128, 0:512],
                            eT_sb[:, t : t + 2, 0:128],
                            kv_sb[:, t : t + 2, gidx * 512 : (gidx + 1) * 512],
                            start=(tp == 0),
                            stop=(tp == T // 2 - 1),
                            perf_mode=DR,
                        )
                    mm.then_inc(s_p, 1)  # s_p = 10+gidx
                # m1 transposes from e_lo [96, 256]: chunk t lives at
                # partitions 32*(t//2), cols 128*(t%2)
                TRM1_A = [9, 9, 9, 9, 15, 9]
                TRM1_D = [0, 0, 0, 0, 0, 6]
                TRM1_BANKS = [ps_s[3], ps_s[0], ps_s[2], ps_s[1], ps_s[3], ps_s[0]]
                for k in range(6):
                    pe.wait_ge(s_a, TRM1_A[k])
                    if TRM1_D[k]:
                        pe.wait_ge(s_d, TRM1_D[k])
                    pb = 32 * (k // 2)
                    cb = 128 * (k % 2)
                    nc.tensor.matmul(
                        TRM1_BANKS[k][0:128, 0:32],
                        e_lo[pb : pb + 32, cb : cb + 128],
                        iden[pb : pb + 32, pb : pb + 32],
                        is_transpose=True,
                        start=True,
                        stop=True,
                    ).then_inc(s_p, 1)  # s_p = 20..25
                # phase 2 m1: rounds of col-tiled n-tiles sharing one psum
                # bank at partition bases 0/32/64/96; the small (2-tile)
                # round g8-9 runs FIRST so the last round's copy+DMA tail is
                # a full-width transfer
                pe.wait_ge(s_a, 17)
                pe.wait_ge(s_d, 8)
                RBANKS = [ps_s[0], ps_t[2], ps_t[3]]
                RG = [(8, 2), (0, 4), (4, 4)]
                for r in range(3):
                    g0, ntile = RG[r]
                    bank = RBANKS[r]
                    for t in range(T):
                        for c in range(ntile):
                            gcol = (g0 + c) * 512
                            mm = nc.tensor.matmul(
                                bank[32 * c : 32 * c + 32, 0:512],
                                eT_sb[:, t, 128:160],
                                kv_sb[:, t, gcol : gcol + 512],
                                start=(t == 0),
                                stop=(t == T - 1),
                                tile_position=(0, 32 * c),
                            )
                    mm.then_inc(s_p, 1)  # s_p = 26+r

            @block.scalar
            def _(act):
                # preload the EXP activation table while idle
                act.wait_ge(s_g, 3)
                nc.scalar.activation(
                    e_hi[0:1, 0:1], bias0[0:1, 0:1], AF.Exp, bias=0.0, scale=1.0
                )
                # build z slab 3 on-chip (slab0 shifted by 3 bytes)
                act.wait_ge(s_z1, 16)
                nc.scalar.activation(
                    z_sb[:, 3, 0 : ZFL - 3], z_sb[:, 0, 3:ZFL], AF.Copy
                ).then_inc(s_z2, 1)

                def expcall(ps, esl, b):
                    # e' = 64 * exp(s): bias ln(64) keeps the centered
                    # f' = e' - 64 in e4m3's well-resolved range.
                    nc.scalar.activation(
                        esl, ps, AF.Exp, bias=b, scale=SCALE
                    ).then_inc(s_a, 1)

                act.wait_ge(s_p, 1)
                expcall(ps_s[0][:, :], e_hi[:, 0:N0], bias0[:, :])        # s_a=1
                act.wait_ge(s_p, 2)
                expcall(ps_s[2][:, :], e_hi[:, OFF1 : OFF1 + N1], bias0[:, :])  # 2
                TRC0_BANKS = [ps_t[0], ps_t[1], ps_t[2], ps_t[3], ps_s[0], ps_s[2]]
                for k in range(6):
                    # transposed m0 chunks -> f' = e' - 64 (cast to e4m3)
                    act.wait_ge(s_p, 3 + k)
                    nc.scalar.activation(
                        eT_sb[:, k, 0:128],
                        TRC0_BANKS[k][0:128, 0:128],
                        AF.Copy,
                        bias=-64.0,
                    ).then_inc(s_a, 1)                                   # s_a=3..8
                act.wait_ge(s_p, 9)
                expcall(ps_s[1][0:96, 0:256], e_lo[0:96, 0:256], bias0[0:96, :])  # 9
                for h in range(5):
                    # ph2-m0 even-g out copies (odd g on the DVE)
                    g2 = 2 * h
                    act.wait_ge(s_p, 10 + g2)
                    nc.scalar.activation(
                        o_hi[:, g2 * 512 : (g2 + 1) * 512],
                        ps_t[g2 % 4][0:128, 0:512],
                        AF.Copy,
                    ).then_inc(s_a, 1)  # s_a = 10+h
                for k, tb in ((0, 3), (2, 2), (4, 3)):
                    act.wait_ge(s_p, 20 + k)
                    nc.scalar.activation(
                        eT_sb[:, k, 128:160],
                        ps_s[tb][0:128, 0:32],
                        AF.Copy,
                        bias=-64.0,
                    ).then_inc(s_a, 1)  # s_a = 15, 16, 17
                act.wait_ge(s_p, 26)
                nc.scalar.activation(
                    o_lo[0:64, 0, :], ps_s[0][0:64, 0:512], AF.Copy
                ).then_inc(s_a, 1)  # s_a = 18: round 0 (g8-9)
                act.wait_ge(s_p, 28)
                nc.scalar.activation(
                    o_lo[:, 2, :], ps_t[3][0:128, 0:512], AF.Copy
                ).then_inc(s_a, 1)  # s_a = 19: round 2 (g4-7)

            @block.vector
            def _(dve):
                # build z slab 2 on-chip (slab0 shifted by 2 bytes)
                dve.wait_ge(s_z1, 16)
                nc.vector.tensor_copy(
                    z_sb[:, 2, 0 : ZFL - 2], z_sb[:, 0, 2:ZFL]
                ).then_inc(s_z2, 1)
                dve.wait_ge(s_a, 2)
                nc.vector.reduce_sum(
                    dh_sb[:], e_hi[:, :], axis=mybir.AxisListType.X
                ).then_inc(s_v, 1)
                # ph2-m0 odd-g out copies
                for h in range(5):
                    g2 = 2 * h + 1
                    dve.wait_ge(s_p, 10 + g2)
                    nc.vector.tensor_copy(
                        o_hi[:, g2 * 512 : (g2 + 1) * 512],
                        ps_t[g2 % 4][0:128, 0:512],
                    ).then_inc(s_d, 1)  # s_d = 1+h
                for k, tb in ((1, 0), (3, 1), (5, 0)):
                    dve.wait_ge(s_p, 20 + k)
                    nc.vector.tensor_scalar_add(
                        eT_sb[:, k, 128:160],
                        ps_s[tb][0:128, 0:32],
                        -64.0,
                    ).then_inc(s_d, 1)  # s_d = 6, 7, 8
                dve.wait_ge(s_a, 9)
                for c in range(3):
                    nc.vector.reduce_sum(
                        dl_sb[32 * c : 32 * c + 32, :],
                        e_lo[32 * c : 32 * c + 32, :],
                        axis=mybir.AxisListType.X,
                    ).then_inc(s_v, 1)  # s_v = 2..4
                dve.wait_ge(s_p, 27)
                nc.vector.tensor_copy(
                    o_lo[:, 1, :], ps_t[2][0:128, 0:512]
                ).then_inc(s_d, 1)  # s_d = 9: ph2-m1 round 1 (g0-3)

    return nc


def _host_prep(z1_hat, z2):
    z1 = np.asarray(z1_hat, dtype=np.float32)[0]  # [128, 100, 64]
    z2a = np.asarray(z2, dtype=np.float32)[0]

    # q patches [160, 5120] and lhsT layout qT3 [128, 40, 160]
    q = z1.reshape(KC, NH, KH, NW, KW).transpose(1, 3, 0, 2, 4).reshape(PQ, D)
    qT3 = np.ascontiguousarray(q.reshape(PQ, KC, KH * KW).transpose(1, 2, 0).astype(F8))

    # padded z2: rows 100..111 zero
    z_pad = np.zeros((KC, 112, W), dtype=np.float32)
    z_pad[:, :H] = z2a

    # sliding kv patches from padded z2
    sw = np.lib.stride_tricks.sliding_window_view(z_pad, (KH, KW), axis=(1, 2))
    # sw: [128, 103, 61, 10, 4]; patch(h, w) = sw[:, h, w]

    q64 = q.astype(np.float64)
    ij_off = (np.arange(KH)[:, None] * W + np.arange(KW)[None, :]).reshape(-1)  # [40]

    in_maps = []
    corrs = []
    for core in range(NCORES):
        h0 = HPC * core
        zf = z_pad[:, h0 : h0 + ZROWS, :].reshape(KC, ZFL)
        # the 4 byte-shifted slabs, quantized once so all shifts agree
        zf8 = np.zeros((KC, ZFL + KW), dtype=F8)
        zf8[:, :ZFL] = zf.astype(F8)
        z4 = np.stack([zf8[:, s : s + ZFL] for s in range(KW)], axis=1)
        # kv rows indexed by flat position p = h_local*64 + w
        kvp = np.zeros((PKC, D), dtype=np.float32)
        hh = np.arange(PKC) // W
        ww = np.arange(PKC) % W
        real = (ww < WK) & (h0 + hh < HK)
        ridx = np.nonzero(real)[0]
        kvp[ridx] = (
            sw[:, h0 + hh[ridx], ww[ridx]].transpose(1, 0, 2, 3).reshape(-1, D)
        )
        kvr = np.ascontiguousarray(
            kvp.reshape(T, 128, D).transpose(1, 0, 2).astype(F8)
        )
        in_maps.append(
            {
                "z4": np.ascontiguousarray(z4),
                "qT3": qT3,
                "kvr": kvr,
            }
        )
        # denominator correction: computed-but-invalid columns (w >= 61 or
        # h >= 91) polluted the on-chip row-sum; subtract their exactly
        # recomputed exp contribution.
        bad = np.nonzero(~real)[0]
        win = zf.astype(np.float64)[:, bad[:, None] + ij_off[None, :]]  # [128,nb,40]
        patches = win.transpose(1, 0, 2).reshape(len(bad), D)  # d-order (c, i, j)
        s_bad = q64 @ patches.T  # [160, nb]
        corrs.append(np.exp(s_bad * SCALE).sum(axis=1))

    corr = np.sum(corrs, axis=0)
    # centered softmax: device returns f' @ kv with f' = 64*(e-1); host adds
    # the exact colsum term sum_k kv[k, :] over all real patches (all cores).
    swr = sw[:, :HK, :WK]
    colsum = swr.astype(np.float64).sum(axis=(1, 2)).reshape(D)  # [5120]
    return in_maps, corr, colsum


def kernel(z1_hat, z2):
    from concourse.bass_utils import run_bass_kernel_spmd

    in_maps, corr, colsum = _host_prep(z1_hat, z2)
    if "nc" not in _CACHE:
        _CACHE["nc"] = _build_nc()
    nc = _CACHE["nc"]
    res = run_bass_kernel_spmd(nc, in_maps, list(range(NCORES)))
    num = np.broadcast_to(colsum, (PQ, D)).astype(np.float64).copy()
    den = -corr
    for r in res.results:
        num[0:128] += r["out"].astype(np.float64) / 64.0
        o2 = r["out2"].astype(np.float64).reshape(4, 32, 3, 512)
        m1 = np.concatenate(
            [
                o2[:, :, 1, :].transpose(1, 0, 2).reshape(32, 2048),
                o2[:, :, 2, :].transpose(1, 0, 2).reshape(32, 2048),
                o2[0:2, :, 0, :].transpose(1, 0, 2).reshape(32, 1024),
            ],
            axis=1,
        )
        num[128:160] += m1 / 64.0
        dv = r["den"].astype(np.float64)[:, 0] / 64.0
        den = den + np.concatenate(
            [dv[0:128], dv[128:160] + dv[160:192] + dv[192:224]]
        )
    out = (num / den[:, None]).astype(np.float32)
    # fold patches back: [160, 5120] -> [1, 128, 100, 64]
    out = out.reshape(NH, NW, KC, KH, KW).transpose(2, 0, 3, 1, 4)
    return np.ascontiguousarray(out.reshape(1, KC, H, W))


# revision 34
# speedup vs baseline: 1.0749x; 1.0541x over previous
"""Trainium2 Bass kernel for BottleneckAttention (patch attention).

q patches [160, 5120] from z1_hat (non-overlapping 10x4 unfold),
kv patches [5551, 5120] from z2 (overlapping unfold, Hk=91 x Wk=61),
scores = q @ kv.T / 5120, softmax over kv patches, out = attn @ kv,
folded back to [1, 128, 100, 64].

Sharding: contiguous blocks of 12 kv h-rows per core (8 x 12 = 96 >= 91).
Each core owns the 768 flat positions p = h_local*64 + w (w in [0,64);
positions with w >= 61 or h >= 91 are invalid -- their kv rows are zeroed
so they never touch the numerator, and the host subtracts their exactly
recomputed exp contribution from the denominator. Every core computes all
160 q rows; the host combines with an all-gather softmax.

rev H (fp8 + DoubleRow + tile packing):
- All matmul operands are float8 e4m3. The M=128 q-row block uses
  perf_mode=DoubleRow (two 128-deep k-tiles per pass); the 32-row m1
  block uses 3-way (phase 1) / 4-way (phase 2) 32-column tile_position
  packing, which runs the small matmuls concurrently in disjoint PE
  column groups (DoubleRow forbids non-zero dst partitions).
- exp applies bias=ln(64) so the device works with e' = 64*exp(s); the
  PSUM->SBUF copies subtract 64, giving f' = 64*(e^s - 1) in e4m3's
  well-resolved range. Outputs leave as fp8 partials; the host divides
  by 64 and adds the exact colsum term (centered softmax).
- All inputs ride ONE DMA queue in priority order (z slabs 0-1, q
  halves, then kv in three blocks) -- a single queue sustains ~340 GB/s
  where contending queues share ~250. z slabs 2-3 are built on-chip by
  the idle Vector/Scalar engines. Output DMAs use the gpsimd queue.
- PE program: warmup (clock/pipeline ramp) | phase-1 m0 (512/256 col
  groups) | m0 transposes | m1 chains | phase-2 m0 | m1 transposes |
  phase-2 m1 in three col-packed rounds (small round first so the tail
  transfer is full-width). PSUM->SBUF out-copies alternate between the
  Scalar and Vector engines so neither paces the PE.
- The m1 output block leaves in device-native (tile,row) layout via a
  second DRAM tensor; the host reorders it.
"""

import sys

sys.path.insert(0, "/opt/trn_rl_repo")

import math

import numpy as np
import ml_dtypes

import concourse.bass as bass
import concourse.mybir as mybir

DT = mybir.dt
AF = mybir.ActivationFunctionType
DR = mybir.MatmulPerfMode.DoubleRow

# problem geometry (hardcoded from the reference module)
KC, KH, KW = 128, 10, 4
H, W = 100, 64
NH, NW = H // KH, W // KW          # 10, 16
PQ = NH * NW                       # 160 q patches
D = KC * KH * KW                   # 5120
HK, WK = H - KH + 1, W - KW + 1    # 91, 61
NCORES = 8
HPC = 12                           # kv h-rows per core (8*12 = 96 >= 91)
PKC = HPC * W                      # 768 flat positions per core
T = 6                              # 768 / 128 k-chunks for phase 2
G0H, G1H = 8, 4                    # phase-1 h-groups (8+4 = 12)
N0 = G0H * W                       # 448: contiguous stream for h 0..6
N1 = G1H * W                       # 320: contiguous stream for h 7..11
OFF1 = G0H * W                     # 448: flat offset of group 1
ZROWS = 2 * HPC                    # 24 z rows staged per core
ZFL = ZROWS * W                    # 1536 flat z cols
NT = D // 512                      # 10 phase-2 n-tiles
SCALE = 1.0 / D
LN64 = math.log(64.0)

F8 = ml_dtypes.float8_e4m3fn

_CACHE = {}


def _build_nc():
    nc = bass.Bass()
    # z4: the 4 byte-shifted slabs, staged by the host (one DMA)
    z_d = nc.declare_dram_parameter("z4", [KC, KW, ZFL], DT.float8e4, isOutput=False)
    q_d = nc.declare_dram_parameter("qT3", [KC, KH * KW, PQ], DT.float8e4, isOutput=False)
    kv_d = nc.declare_dram_parameter("kvr", [128, T, D], DT.float8e4, isOutput=False)
    out_d = nc.declare_dram_parameter("out", [128, D], DT.float8e4, isOutput=True)
    out2_d = nc.declare_dram_parameter("out2", [128, 3 * 512], DT.float8e4, isOutput=True)
    den_d = nc.declare_dram_parameter("den", [PQ + 96, 1], DT.float32, isOutput=True)

    from contextlib import ExitStack

    ctx = ExitStack()
    with ctx:
        z_sb = ctx.enter_context(nc.sbuf_tensor([KC, KW, ZFL], DT.float8e4))
        q_sb = ctx.enter_context(nc.sbuf_tensor([KC, KH * KW, PQ], DT.float8e4))
        kv_sb = ctx.enter_context(nc.sbuf_tensor([128, T, D], DT.float8e4))
        e_hi = ctx.enter_context(nc.sbuf_tensor([128, PKC], DT.float32))
        e_lo = ctx.enter_context(nc.sbuf_tensor([96, 256], DT.float32))
        eT_sb = ctx.enter_context(nc.sbuf_tensor([128, T, PQ], DT.float8e4))
        o_hi = ctx.enter_context(nc.sbuf_tensor([128, D], DT.float8e4))
        o_lo = ctx.enter_context(nc.sbuf_tensor([128, 3, 512], DT.float8e4))
        iden = ctx.enter_context(nc.sbuf_tensor([128, 128], DT.float32))
        wz = ctx.enter_context(nc.sbuf_tensor([128, 512], DT.bfloat16))
        bias0 = ctx.enter_context(nc.sbuf_tensor([128, 1], DT.float32))
        dh_sb = ctx.enter_context(nc.sbuf_tensor([128, 1], DT.float32))
        dl_sb = ctx.enter_context(nc.sbuf_tensor([96, 1], DT.float32))

        # phase-1 score accumulators: (h-group, q-half)
        ps_s = [
            ctx.enter_context(nc.psum_tensor(f"ps_s{i}", [128, n], DT.float32))
            for i, n in enumerate((N0, N0, N1, 384))
        ]  # order: g0m0, g0m1, g1m0, g1m1
        # transpose staging / phase-2 accumulators (4 distinct banks)
        ps_t = [
            ctx.enter_context(nc.psum_tensor(f"ps_t{i}", [128, 512], DT.float32))
            for i in range(4)
        ]

        s_z1 = ctx.enter_context(nc.semaphore("s_z1"))
        s_z2 = ctx.enter_context(nc.semaphore("s_z2"))
        s_q0 = ctx.enter_context(nc.semaphore("s_q0"))
        s_q1 = ctx.enter_context(nc.semaphore("s_q1"))
        s_kv1 = ctx.enter_context(nc.semaphore("s_kv1"))
        s_kv2 = ctx.enter_context(nc.semaphore("s_kv2"))
        s_kv3 = ctx.enter_context(nc.semaphore("s_kv3"))
        s_p = ctx.enter_context(nc.semaphore("s_p"))
        s_a = ctx.enter_context(nc.semaphore("s_a"))
        s_v = ctx.enter_context(nc.semaphore("s_v"))
        s_d = ctx.enter_context(nc.semaphore("s_d"))
        s_g = ctx.enter_context(nc.semaphore("s_g"))
        s_o = ctx.enter_context(nc.semaphore("s_o"))

        # rev D schedule:
        #   PE:  warmup | g0m0 | g1m0 | tr_m0 | m1 (3-way col-tiled) |
        #        ph2-m0 (DR) | tr_m1 | ph2-m1 (col-tiled rounds)
        #   s_p: 1 g0m0 | 2 g1m0 | 3-8 tr_m0 | 9 m1 | 10-19 ph2m0 |
        #        20-25 tr_m1 | 26-28 ph2m1 rounds
        #   s_a (ACT): 1 exp0 | 2 exp1 | 3-8 trcopy_m0 | 9 exp_m1 |
        #        10-14 ph2m0 even copies | 15-20 trcopy_m1 | 21-23 round copies
        #   s_d (DVE): 1-5 ph2m0 odd copies;  s_v: 1 dh | 2-4 dl

        with nc.Block() as block:

            @block.gpsimd
            def _(g):
                g.memset(wz[:], 0.0).then_inc(s_g, 1)        # 1: warmup tile
                g.memset(iden[:], 0.0)
                g.affine_select(
                    out=iden[:],
                    in_=iden[:],
                    compare_op=mybir.AluOpType.not_equal,
                    fill=1.0,
                    base=0,
                    pattern=[[-1, 128]],
                    channel_multiplier=1,
                ).then_inc(s_g, 1)                            # 2: identity
                g.memset(bias0[:], LN64).then_inc(s_g, 1)     # 3: exp bias
                # out m0 halves behind the ACT/DVE psum->sbuf copies
                g.wait_ge(s_a, 15)
                g.wait_ge(s_d, 6)
                g.dma_start(out_d[:, 0:2560], o_hi[:, 0:2560]).then_inc(s_o, 16)
                g.wait_ge(s_a, 17)
                g.wait_ge(s_d, 8)
                g.dma_start(out_d[:, 2560:], o_hi[:, 2560:]).then_inc(s_o, 16)
                # q-rows 128..159 output in device-native (c,row) layout;
                # one DMA per phase-2-m1 round, right after its copy
                g.wait_ge(s_a, 18)
                g.dma_start(out2_d[0:64, 0:512], o_lo[0:64, 0, :]).then_inc(s_o, 16)
                g.wait_ge(s_d, 9)
                g.dma_start(out2_d[:, 512:1024], o_lo[:, 1, :]).then_inc(s_o, 16)
                g.wait_ge(s_a, 19)
                g.dma_start(out2_d[:, 1024:1536], o_lo[:, 2, :]).then_inc(s_o, 16)

            @block.sync
            def _(sync):
                # ALL inputs on one queue in priority order: a single queue
                # sustains ~340 GB/s while two contending queues drop to ~250
                sync.dma_start(z_sb[:, 0:2, :], z_d[:, 0:2, :]).then_inc(s_z1, 16)
                sync.dma_start(q_sb[:, 0:20, :], q_d[:, 0:20, :]).then_inc(s_q0, 16)
                sync.dma_start(q_sb[:, 20:40, :], q_d[:, 20:40, :]).then_inc(
                    s_q1, 16
                )
                sync.dma_start(kv_sb[:, :, 0:1536], kv_d[:, :, 0:1536]).then_inc(
                    s_kv1, 16
                )
                sync.dma_start(
                    kv_sb[:, :, 1536:3584], kv_d[:, :, 1536:3584]
                ).then_inc(s_kv2, 16)
                sync.dma_start(
                    kv_sb[:, :, 3584:5120], kv_d[:, :, 3584:5120]
                ).then_inc(s_kv3, 16)
                sync.wait_ge(s_v, 1)
                sync.dma_start(den_d[0:128, :], dh_sb[:]).then_inc(s_o, 16)
                sync.wait_ge(s_v, 4)
                sync.dma_start(den_d[128:224, :], dl_sb[:]).then_inc(s_o, 16)
                sync.wait_ge(s_o, 112)

            @block.tensor
            def _(pe):
                # HAM warmup on the zeroed bf16 tile while input DMAs land.
                pe.wait_ge(s_g, 1)
                for _w in range(5):
                    nc.tensor.matmul(
                        ps_t[0][0:128, 0:512],
                        wz[:, 0:128],
                        wz[:, 0:512],
                        start=(_w == 0),
                        stop=(_w == 4),
                    )
                for _w in range(5):
                    nc.tensor.matmul(
                        ps_t[1][0:128, 0:256],
                        wz[:, 0:128],
                        wz[:, 0:256],
                        start=(_w == 0),
                        stop=(_w == 4),
                    )
                # phase 1 m0: scores[pq0:128, pos] -- DR over (j, j+1) pairs.
                # Step order: all jp=0 pairs (slabs 0-1, q half 0) first, then
                # jp=2 (slabs 2-3, q half 1). Junk cols corrected on host.
                def chain(grp, first):
                    ps = ps_s[grp * 2]
                    for s in range(20):
                        if first and s == 0:
                            pe.wait_ge(s_z1, 16)
                            pe.wait_ge(s_q0, 16)
                        if first and s == 5:
                            pe.wait_ge(s_z2, 2)
                        if first and s == 10:
                            pe.wait_ge(s_q1, 16)
                        blk = s // 5
                        i_, jp = (blk // 2) * 5 + s % 5, (blk % 2) * 2
                        ij = i_ * KW + jp
                        st, sp = s == 0, s == 19
                        base = i_ * W + (OFF1 if grp == 1 else 0)
                        n = N1 if grp == 1 else N0
                        mm = nc.tensor.matmul(
                            ps[:, :],
                            q_sb[:, ij : ij + 2, 0:128],
                            z_sb[:, jp : jp + 2, base : base + n],
                            start=st,
                            stop=sp,
                            perf_mode=DR,
                        )
                    return mm

                chain(0, True).then_inc(s_p, 1)   # s_p = 1
                chain(1, False).then_inc(s_p, 1)  # s_p = 2
                # m0 transposes (fp32): bank k%4; banks 0/1 reused at k=4,5
                pe.wait_ge(s_g, 2)
                TRM0_FREED = [1, 1, 1, 1, 2, 2]
                TRM0_BANKS = [ps_t[0], ps_t[1], ps_t[2], ps_t[3], ps_s[0], ps_s[2]]
                for k in range(6):
                    pe.wait_ge(s_a, TRM0_FREED[k])
                    nc.tensor.matmul(
                        TRM0_BANKS[k][0:128, 0:128],
                        e_hi[:, k * 128 : (k + 1) * 128],
                        iden[0:128, 0:128],
                        is_transpose=True,
                        start=True,
                        stop=True,
                    ).then_inc(s_p, 1)               # s_p = 3..8
                # m1 score chains (q rows 128:160): 3-way col tiling, each
                # tile owns 256 positions; plain fp8 (no DR -- col tiling and
                # DR are mutually exclusive)
                for s in range(40):
                    i_, jp = s % 10, (s // 10) % 2 * 2
                    j_ = jp + (0 if s < 20 else 1)
                    ij = i_ * KW + j_
                    st, sp_ = s == 0 or s == 1, s == 38 or s == 39
                    for c in range(3):
                        mm = nc.tensor.matmul(
                            ps_s[1][32 * c : 32 * c + 32, 0:256],
                            q_sb[:, ij, 128:160],
                            z_sb[:, j_, i_ * W + 256 * c : i_ * W + 256 * c + 256],
                            start=(s == 0),
                            stop=(s == 39),
                            tile_position=(0, 32 * c),
                        )
                mm.then_inc(s_p, 1)  # s_p = 9
                TRM1_A = [9, 9, 9, 9, 10, 0]
                TRM1_D = [0, 0, 0, 0, 0, 1]
                TRM1_BANKS = [ps_s[3], ps_s[0], ps_s[2], ps_s[1], ps_s[3], ps_s[0]]
                for k in range(6):
                    if TRM1_A[k]:
                        pe.wait_ge(s_a, TRM1_A[k])
                    if TRM1_D[k]:
                        pe.wait_ge(s_d, TRM1_D[k])
                    pb = 32 * (k // 2)
                    cb = 128 * (k % 2)
                    nc.tensor.matmul(
                        TRM1_BANKS[k][0:128, 0:32],
                        e_lo[pb : pb + 32, cb : cb + 128],
                        iden[pb : pb + 32, pb : pb + 32],
                        is_transpose=True,
                        start=True,
                        stop=True,
                    ).then_inc(s_p, 1)  # s_p = 10..15
                # phase 2 m0 (DR over k-chunk pairs)
                pe.wait_ge(s_a, 8)
                PH2_BANK_FREE = [(0, 0), (0, 0), (0, 0), (0, 0),
                                 (1, 13), (2, 4), (1, 14), (2, 5),
                                 (1, 15), (2, 6)]
                for gidx in range(NT):
                    if gidx == 0:
                        pe.wait_ge(s_kv1, 16)
                    elif gidx == 3:
                        pe.wait_ge(s_kv2, 16)
                    elif gidx == 7:
                        pe.wait_ge(s_kv3, 16)
                    kind, val = PH2_BANK_FREE[gidx]
                    if kind == 1:
                        pe.wait_ge(s_a, val)
                    elif kind == 2:
                        pe.wait_ge(s_d, val)
                    for tp in range(T // 2):
                        t = 2 * tp
                        mm = nc.tensor.matmul(
                            ps_t[gidx % 4][0:128, 0:512],
                            eT_sb[:, t : t + 2, 0:128],
                            kv_sb[:, t : t + 2, gidx * 512 : (gidx + 1) * 512],
                            start=(tp == 0),
                            stop=(tp == T // 2 - 1),
                            perf_mode=DR,
                        )
                    mm.then_inc(s_p, 1)  # s_p = 16+gidx
                # m1 transposes from e_lo [96, 256]: chunk t lives at
                # partitions 32*(t//2), cols 128*(t%2)
                # phase 2 m1: rounds of col-tiled n-tiles sharing one psum
                # bank at partition bases 0/32/64/96; the small (2-tile)
                # round g8-9 runs FIRST so the last round's copy+DMA tail is
                # a full-width transfer
                pe.wait_ge(s_a, 12)
                pe.wait_ge(s_d, 3)
                RBANKS = [ps_s[0], ps_t[2], ps_t[3]]
                RWAIT = [(0, 0), (1, 16), (2, 7)]
                RG = [(8, 2), (0, 4), (4, 4)]
                for r in range(3):
                    g0, ntile = RG[r]
                    bank = RBANKS[r]
                    kind, val = RWAIT[r]
                    if kind == 1:
                        pe.wait_ge(s_a, val)
                    elif kind == 2:
                        pe.wait_ge(s_d, val)
                    for t in range(T):
                        for c in range(ntile):
                            gcol = (g0 + c) * 512
                            mm = nc.tensor.matmul(
                                bank[32 * c : 32 * c + 32, 0:512],
                                eT_sb[:, t, 128:160],
                                kv_sb[:, t, gcol : gcol + 512],
                                start=(t == 0),
                                stop=(t == T - 1),
                                tile_position=(0, 32 * c),
                            )
                    mm.then_inc(s_p, 1)  # s_p = 26+r

            @block.scalar
            def _(act):
                # preload the EXP activation table while idle
                act.wait_ge(s_g, 3)
                nc.scalar.activation(
                    e_hi[0:1, 0:1], bias0[0:1, 0:1], AF.Exp, bias=0.0, scale=1.0
                )
                # build z slab 3 on-chip (slab0 shifted by 3 bytes)
                act.wait_ge(s_z1, 16)
                nc.scalar.activation(
                    z_sb[:, 3, 0 : ZFL - 3], z_sb[:, 0, 3:ZFL], AF.Copy
                ).then_inc(s_z2, 1)

                def expcall(ps, esl, b):
                    # e' = 64 * exp(s): bias ln(64) keeps the centered
                    # f' = e' - 64 in e4m3's well-resolved range.
                    nc.scalar.activation(
                        esl, ps, AF.Exp, bias=b, scale=SCALE
                    ).then_inc(s_a, 1)

                act.wait_ge(s_p, 1)
                expcall(ps_s[0][:, :], e_hi[:, 0:N0], bias0[:, :])        # s_a=1
                act.wait_ge(s_p, 2)
                expcall(ps_s[2][:, :], e_hi[:, OFF1 : OFF1 + N1], bias0[:, :])  # 2
                TRC0_BANKS = [ps_t[0], ps_t[1], ps_t[2], ps_t[3], ps_s[0], ps_s[2]]
                for k in range(6):
                    # transposed m0 chunks -> f' = e' - 64 (cast to e4m3)
                    act.wait_ge(s_p, 3 + k)
                    nc.scalar.activation(
                        eT_sb[:, k, 0:128],
                        TRC0_BANKS[k][0:128, 0:128],
                        AF.Copy,
                        bias=-64.0,
                    ).then_inc(s_a, 1)                                   # s_a=3..8
                act.wait_ge(s_p, 9)
                expcall(ps_s[1][0:96, 0:256], e_lo[0:96, 0:256], bias0[0:96, :])  # 9
                for k, tb in ((0, 3), (2, 2), (4, 3)):
                    act.wait_ge(s_p, 10 + k)
                    nc.scalar.activation(
                        eT_sb[:, k, 128:160],
                        ps_s[tb][0:128, 0:32],
                        AF.Copy,
                        bias=-64.0,
                    ).then_inc(s_a, 1)  # s_a = 10, 11, 12
                for h in range(5):
                    # ph2-m0 even-g out copies (odd g on the DVE)
                    g2 = 2 * h
                    act.wait_ge(s_p, 16 + g2)
                    nc.scalar.activation(
                        o_hi[:, g2 * 512 : (g2 + 1) * 512],
                        ps_t[g2 % 4][0:128, 0:512],
                        AF.Copy,
                    ).then_inc(s_a, 1)  # s_a = 13+h
                act.wait_ge(s_p, 26)
                nc.scalar.activation(
                    o_lo[0:64, 0, :], ps_s[0][0:64, 0:512], AF.Copy
                ).then_inc(s_a, 1)  # s_a = 18: round 0 (g8-9)
                act.wait_ge(s_p, 28)
                nc.scalar.activation(
                    o_lo[:, 2, :], ps_t[3][0:128, 0:512], AF.Copy
                ).then_inc(s_a, 1)  # s_a = 19: round 2 (g4-7)

            @block.vector
            def _(dve):
                # build z slab 2 on-chip (slab0 shifted by 2 bytes)
                dve.wait_ge(s_z1, 16)
                nc.vector.tensor_copy(
                    z_sb[:, 2, 0 : ZFL - 2], z_sb[:, 0, 2:ZFL]
                ).then_inc(s_z2, 1)
                dve.wait_ge(s_a, 2)
                nc.vector.reduce_sum(
                    dh_sb[:], e_hi[:, :], axis=mybir.AxisListType.X
                ).then_inc(s_v, 1)
                # ph2-m0 odd-g out copies
                for k, tb in ((1, 0), (3, 1), (5, 0)):
                    dve.wait_ge(s_p, 10 + k)
                    nc.vector.tensor_scalar_add(
                        eT_sb[:, k, 128:160],
                        ps_s[tb][0:128, 0:32],
                        -64.0,
                    ).then_inc(s_d, 1)  # s_d = 1, 2, 3
                dve.wait_ge(s_a, 9)
                for c in range(3):
                    nc.vector.reduce_sum(
                        dl_sb[32 * c : 32 * c + 32, :],
                        e_lo[32 * c : 32 * c + 32, :],
                        axis=mybir.AxisListType.X,
                    ).then_inc(s_v, 1)  # s_v = 2..4
                for h in range(5):
                    g2 = 2 * h + 1
                    dve.wait_ge(s_p, 16 + g2)
                    nc.vector.tensor_copy(
                        o_hi[:, g2 * 512 : (g2 + 1) * 512],
                        ps_t[g2 % 4][0:128, 0:512],
                    ).then_inc(s_d, 1)  # s_d = 4+h
                dve.wait_ge(s_p, 27)
                nc.vector.tensor_copy(
                    o_lo[:, 1, :], ps_t[2][0:128, 0:512]
                ).then_inc(s_d, 1)  # s_d = 9: ph2-m1 round 1 (g0-3)

    return nc


def _host_prep(z1_hat, z2):
    z1 = np.asarray(z1_hat, dtype=np.float32)[0]  # [128, 100, 64]
    z2a = np.asarray(z2, dtype=np.float32)[0]

    # q patches [160, 5120] and lhsT layout qT3 [128, 40, 160]
    q = z1.reshape(KC, NH, KH, NW, KW).transpose(1, 3, 0, 2, 4).reshape(PQ, D)
    qT3 = np.ascontiguousarray(q.reshape(PQ, KC, KH * KW).transpose(1, 2, 0).astype(F8))

    # padded z2: rows 100..111 zero
    z_pad = np.zeros((KC, 112, W), dtype=np.float32)
    z_pad[:, :H] = z2a

    # sliding kv patches from padded z2
    sw = np.lib.stride_tricks.sliding_window_view(z_pad, (KH, KW), axis=(1, 2))
    # sw: [128, 103, 61, 10, 4]; patch(h, w) = sw[:, h, w]

    q64 = q.astype(np.float64)
    ij_off = (np.arange(KH)[:, None] * W + np.arange(KW)[None, :]).reshape(-1)  # [40]

    in_maps = []
    corrs = []
    for core in range(NCORES):
        h0 = HPC * core
        zf = z_pad[:, h0 : h0 + ZROWS, :].reshape(KC, ZFL)
        # the 4 byte-shifted slabs, quantized once so all shifts agree
        zf8 = np.zeros((KC, ZFL + KW), dtype=F8)
        zf8[:, :ZFL] = zf.astype(F8)
        z4 = np.stack([zf8[:, s : s + ZFL] for s in range(KW)], axis=1)
        # kv rows indexed by flat position p = h_local*64 + w
        kvp = np.zeros((PKC, D), dtype=np.float32)
        hh = np.arange(PKC) // W
        ww = np.arange(PKC) % W
        real = (ww < WK) & (h0 + hh < HK)
        ridx = np.nonzero(real)[0]
        kvp[ridx] = (
            sw[:, h0 + hh[ridx], ww[ridx]].transpose(1, 0, 2, 3).reshape(-1, D)
        )
        kvr = np.ascontiguousarray(
            kvp.reshape(T, 128, D).transpose(1, 0, 2).astype(F8)
        )
        in_maps.append(
            {
                "z4": np.ascontiguousarray(z4),
                "qT3": qT3,
                "kvr": kvr,
            }
        )
        # denominator correction: computed-but-invalid columns (w >= 61 or
        # h >= 91) polluted the on-chip row-sum; subtract their exactly
        # recomputed exp contribution.
        bad = np.nonzero(~real)[0]
        win = zf.astype(np.float64)[:, bad[:, None] + ij_off[None, :]]  # [128,nb,40]
        patches = win.transpose(1, 0, 2).reshape(len(bad), D)  # d-order (c, i, j)
        s_bad = q64 @ patches.T  # [160, nb]
        corrs.append(np.exp(s_bad * SCALE).sum(axis=1))

    corr = np.sum(corrs, axis=0)
    # centered softmax: device returns f' @ kv with f' = 64*(e-1); host adds
    # the exact colsum term sum_k kv[k, :] over all real patches (all cores).
    swr = sw[:, :HK, :WK]
    colsum = swr.astype(np.float64).sum(axis=(1, 2)).reshape(D)  # [5120]
    return in_maps, corr, colsum


def kernel(z1_hat, z2):
    from concourse.bass_utils import run_bass_kernel_spmd

    in_maps, corr, colsum = _host_prep(z1_hat, z2)
    if "nc" not in _CACHE:
        _CACHE["nc"] = _build_nc()
    nc = _CACHE["nc"]
    res = run_bass_kernel_spmd(nc, in_maps, list(range(NCORES)))
    num = np.broadcast_to(colsum, (PQ, D)).astype(np.float64).copy()
    den = -corr
    for r in res.results:
        num[0:128] += r["out"].astype(np.float64) / 64.0
        o2 = r["out2"].astype(np.float64).reshape(4, 32, 3, 512)
        m1 = np.concatenate(
            [
                o2[:, :, 1, :].transpose(1, 0, 2).reshape(32, 2048),
                o2[:, :, 2, :].transpose(1, 0, 2).reshape(32, 2048),
                o2[0:2, :, 0, :].transpose(1, 0, 2).reshape(32, 1024),
            ],
            axis=1,
        )
        num[128:160] += m1 / 64.0
        dv = r["den"].astype(np.float64)[:, 0] / 64.0
        den = den + np.concatenate(
            [dv[0:128], dv[128:160] + dv[160:192] + dv[192:224]]
        )
    out = (num / den[:, None]).astype(np.float32)
    # fold patches back: [160, 5120] -> [1, 128, 100, 64]
    out = out.reshape(NH, NW, KC, KH, KW).transpose(2, 0, 3, 1, 4)
    return np.ascontiguousarray(out.reshape(1, KC, H, W))


# revision 35
# speedup vs baseline: 1.1328x; 1.0539x over previous
"""Trainium2 Bass kernel for BottleneckAttention (patch attention).

q patches [160, 5120] from z1_hat (non-overlapping 10x4 unfold),
kv patches [5551, 5120] from z2 (overlapping unfold, Hk=91 x Wk=61),
scores = q @ kv.T / 5120, softmax over kv patches, out = attn @ kv,
folded back to [1, 128, 100, 64].

Sharding: contiguous blocks of 12 kv h-rows per core (8 x 12 = 96 >= 91).
Each core owns the 768 flat positions p = h_local*64 + w (w in [0,64);
positions with w >= 61 or h >= 91 are invalid -- their kv rows are zeroed
so they never touch the numerator, and the host subtracts their exactly
recomputed exp contribution from the denominator. Every core computes all
160 q rows; the host combines with an all-gather softmax.

rev H (fp8 + DoubleRow + tile packing):
- All matmul operands are float8 e4m3. The M=128 q-row block uses
  perf_mode=DoubleRow (two 128-deep k-tiles per pass); the 32-row m1
  block uses 3-way (phase 1) / 4-way (phase 2) 32-column tile_position
  packing, which runs the small matmuls concurrently in disjoint PE
  column groups (DoubleRow forbids non-zero dst partitions).
- exp applies bias=ln(64) so the device works with e' = 64*exp(s); the
  PSUM->SBUF copies subtract 64, giving f' = 64*(e^s - 1) in e4m3's
  well-resolved range. Outputs leave as fp8 partials; the host divides
  by 64 and adds the exact colsum term (centered softmax).
- All inputs ride ONE DMA queue in priority order (z slabs 0-1, q
  halves, then kv in three blocks) -- a single queue sustains ~340 GB/s
  where contending queues share ~250. z slabs 2-3 are built on-chip by
  the idle Vector/Scalar engines. Output DMAs use the gpsimd queue.
- PE program: warmup (clock/pipeline ramp) | phase-1 m0 (512/256 col
  groups) | m0 transposes | m1 chains | phase-2 m0 | m1 transposes |
  phase-2 m1 in three col-packed rounds (small round first so the tail
  transfer is full-width). PSUM->SBUF out-copies alternate between the
  Scalar and Vector engines so neither paces the PE.
- The m1 output block leaves in device-native (tile,row) layout via a
  second DRAM tensor; the host reorders it.
"""

import sys

sys.path.insert(0, "/opt/trn_rl_repo")

import math

import numpy as np
import ml_dtypes

import concourse.bass as bass
import concourse.mybir as mybir

DT = mybir.dt
AF = mybir.ActivationFunctionType
DR = mybir.MatmulPerfMode.DoubleRow

# problem geometry (hardcoded from the reference module)
KC, KH, KW = 128, 10, 4
H, W = 100, 64
NH, NW = H // KH, W // KW          # 10, 16
PQ = NH * NW                       # 160 q patches
D = KC * KH * KW                   # 5120
HK, WK = H - KH + 1, W - KW + 1    # 91, 61
NCORES = 8
HPC = 12                           # kv h-rows per core (8*12 = 96 >= 91)
PKC = HPC * W                      # 768 flat positions per core
T = 6                              # 768 / 128 k-chunks for phase 2
G0H, G1H = 8, 4                    # phase-1 h-groups (8+4 = 12)
N0 = G0H * W                       # 448: contiguous stream for h 0..6
N1 = G1H * W                       # 320: contiguous stream for h 7..11
OFF1 = G0H * W                     # 448: flat offset of group 1
ZROWS = 2 * HPC                    # 24 z rows staged per core
ZFL = ZROWS * W                    # 1536 flat z cols
NT = D // 512                      # 10 phase-2 n-tiles
SCALE = 1.0 / D
LN64 = math.log(64.0)

F8 = ml_dtypes.float8_e4m3fn

_CACHE = {}


def _build_nc():
    nc = bass.Bass()
    # z4: the 4 byte-shifted slabs, staged by the host (one DMA)
    z_d = nc.declare_dram_parameter("z4", [KC, KW, ZFL], DT.float8e4, isOutput=False)
    q_d = nc.declare_dram_parameter("qT3", [KC, KH * KW, PQ], DT.float8e4, isOutput=False)
    kv_d = nc.declare_dram_parameter("kvr", [128, T, D], DT.float8e4, isOutput=False)
    out_d = nc.declare_dram_parameter("out", [128, D], DT.float8e4, isOutput=True)
    out2_d = nc.declare_dram_parameter("out2", [128, 3 * 512], DT.float8e4, isOutput=True)
    den_d = nc.declare_dram_parameter("den", [PQ + 96, 1], DT.float32, isOutput=True)

    from contextlib import ExitStack

    ctx = ExitStack()
    with ctx:
        z_sb = ctx.enter_context(nc.sbuf_tensor([KC, KW, ZFL], DT.float8e4))
        q_sb = ctx.enter_context(nc.sbuf_tensor([KC, KH * KW, PQ], DT.float8e4))
        kv_sb = ctx.enter_context(nc.sbuf_tensor([128, T, D], DT.float8e4))
        e_hi = ctx.enter_context(nc.sbuf_tensor([128, PKC], DT.float32))
        e_lo = ctx.enter_context(nc.sbuf_tensor([96, 256], DT.float32))
        eT_sb = ctx.enter_context(nc.sbuf_tensor([128, T, PQ], DT.float8e4))
        o_hi = ctx.enter_context(nc.sbuf_tensor([128, D], DT.float8e4))
        o_lo = ctx.enter_context(nc.sbuf_tensor([128, 3, 512], DT.float8e4))
        iden = ctx.enter_context(nc.sbuf_tensor([128, 128], DT.float32))
        wz = ctx.enter_context(nc.sbuf_tensor([128, 512], DT.bfloat16))
        bias0 = ctx.enter_context(nc.sbuf_tensor([128, 1], DT.float32))
        dh_sb = ctx.enter_context(nc.sbuf_tensor([128, 1], DT.float32))
        dl_sb = ctx.enter_context(nc.sbuf_tensor([96, 1], DT.float32))

        # phase-1 score accumulators: (h-group, q-half)
        ps_s = [
            ctx.enter_context(nc.psum_tensor(f"ps_s{i}", [128, n], DT.float32))
            for i, n in enumerate((N0, N0, N1, 384))
        ]  # order: g0m0, g0m1, g1m0, g1m1
        # transpose staging / phase-2 accumulators (4 distinct banks)
        ps_t = [
            ctx.enter_context(nc.psum_tensor(f"ps_t{i}", [128, 512], DT.float32))
            for i in range(4)
        ]

        s_z1 = ctx.enter_context(nc.semaphore("s_z1"))
        s_z2 = ctx.enter_context(nc.semaphore("s_z2"))
        s_q0 = ctx.enter_context(nc.semaphore("s_q0"))
        s_q1 = ctx.enter_context(nc.semaphore("s_q1"))
        s_kv1 = ctx.enter_context(nc.semaphore("s_kv1"))
        s_kv2 = ctx.enter_context(nc.semaphore("s_kv2"))
        s_kv3 = ctx.enter_context(nc.semaphore("s_kv3"))
        s_p = ctx.enter_context(nc.semaphore("s_p"))
        s_a = ctx.enter_context(nc.semaphore("s_a"))
        s_v = ctx.enter_context(nc.semaphore("s_v"))
        s_d = ctx.enter_context(nc.semaphore("s_d"))
        s_g = ctx.enter_context(nc.semaphore("s_g"))
        s_o = ctx.enter_context(nc.semaphore("s_o"))

        # rev D schedule:
        #   PE:  warmup | g0m0 | g1m0 | tr_m0 | m1 (3-way col-tiled) |
        #        ph2-m0 (DR) | tr_m1 | ph2-m1 (col-tiled rounds)
        #   s_p: 1 g0m0 | 2 g1m0 | 3-8 tr_m0 | 9 m1 | 10-19 ph2m0 |
        #        20-25 tr_m1 | 26-28 ph2m1 rounds
        #   s_a (ACT): 1 exp0 | 2 exp1 | 3-8 trcopy_m0 | 9 exp_m1 |
        #        10-14 ph2m0 even copies | 15-20 trcopy_m1 | 21-23 round copies
        #   s_d (DVE): 1-5 ph2m0 odd copies;  s_v: 1 dh | 2-4 dl

        with nc.Block() as block:

            @block.gpsimd
            def _(g):
                g.memset(wz[:], 0.0).then_inc(s_g, 1)        # 1: warmup tile
                g.memset(iden[:], 0.0)
                g.affine_select(
                    out=iden[:],
                    in_=iden[:],
                    compare_op=mybir.AluOpType.not_equal,
                    fill=1.0,
                    base=0,
                    pattern=[[-1, 128]],
                    channel_multiplier=1,
                ).then_inc(s_g, 1)                            # 2: identity
                g.memset(bias0[:], LN64).then_inc(s_g, 1)     # 3: exp bias
                # out m0 halves behind the ACT/DVE psum->sbuf copies
                g.wait_ge(s_a, 15)
                g.wait_ge(s_d, 6)
                g.dma_start(out_d[:, 0:2560], o_hi[:, 0:2560]).then_inc(s_o, 16)
                g.wait_ge(s_a, 17)
                g.wait_ge(s_d, 8)
                g.dma_start(out_d[:, 2560:], o_hi[:, 2560:]).then_inc(s_o, 16)
                # q-rows 128..159 output in device-native (c,row) layout;
                # one DMA per phase-2-m1 round, right after its copy
                g.wait_ge(s_a, 18)
                g.dma_start(out2_d[0:64, 0:512], o_lo[0:64, 0, :]).then_inc(s_o, 16)
                g.wait_ge(s_d, 9)
                g.dma_start(out2_d[:, 512:1024], o_lo[:, 1, :]).then_inc(s_o, 16)
                g.wait_ge(s_a, 19)
                g.dma_start(out2_d[:, 1024:1536], o_lo[:, 2, :]).then_inc(s_o, 16)

            @block.sync
            def _(sync):
                # ALL inputs on one queue in priority order: a single queue
                # sustains ~340 GB/s while two contending queues drop to ~250
                sync.dma_start(z_sb[:, 0:2, :], z_d[:, 0:2, :]).then_inc(s_z1, 16)
                sync.dma_start(q_sb[:, 0:20, :], q_d[:, 0:20, :]).then_inc(s_q0, 16)
                sync.dma_start(q_sb[:, 20:40, :], q_d[:, 20:40, :]).then_inc(
                    s_q1, 16
                )
                sync.dma_start(kv_sb[:, :, 0:1536], kv_d[:, :, 0:1536]).then_inc(
                    s_kv1, 16
                )
                sync.dma_start(
                    kv_sb[:, :, 1536:3584], kv_d[:, :, 1536:3584]
                ).then_inc(s_kv2, 16)
                sync.dma_start(
                    kv_sb[:, :, 3584:5120], kv_d[:, :, 3584:5120]
                ).then_inc(s_kv3, 16)
                sync.wait_ge(s_v, 1)
                sync.dma_start(den_d[0:128, :], dh_sb[:]).then_inc(s_o, 16)
                sync.wait_ge(s_v, 4)
                sync.dma_start(den_d[128:224, :], dl_sb[:]).then_inc(s_o, 16)
                sync.wait_ge(s_o, 112)

            @block.tensor
            def _(pe):
                # HAM warmup on the zeroed bf16 tile while input DMAs land.
                pe.wait_ge(s_g, 1)
                for _w in range(5):
                    nc.tensor.matmul(
                        ps_t[0][0:128, 0:512],
                        wz[:, 0:128],
                        wz[:, 0:512],
                        start=(_w == 0),
                        stop=(_w == 4),
                    )
                for _w in range(5):
                    nc.tensor.matmul(
                        ps_t[1][0:128, 0:256],
                        wz[:, 0:128],
                        wz[:, 0:256],
                        start=(_w == 0),
                        stop=(_w == 4),
                    )
                # phase 1 m0: scores[pq0:128, pos] -- DR over (j, j+1) pairs.
                # Step order: all jp=0 pairs (slabs 0-1, q half 0) first, then
                # jp=2 (slabs 2-3, q half 1). Junk cols corrected on host.
                def chain(grp, first):
                    ps = ps_s[grp * 2]
                    for s in range(20):
                        if first and s == 0:
                            pe.wait_ge(s_z1, 16)
                            pe.wait_ge(s_q0, 16)
                        if first and s == 5:
                            pe.wait_ge(s_z2, 2)
                        if first and s == 10:
                            pe.wait_ge(s_q1, 16)
                        blk = s // 5
                        i_, jp = (blk // 2) * 5 + s % 5, (blk % 2) * 2
                        ij = i_ * KW + jp
                        st, sp = s == 0, s == 19
                        base = i_ * W + (OFF1 if grp == 1 else 0)
                        n = N1 if grp == 1 else N0
                        mm = nc.tensor.matmul(
                            ps[:, :],
                            q_sb[:, ij : ij + 2, 0:128],
                            z_sb[:, jp : jp + 2, base : base + n],
                            start=st,
                            stop=sp,
                            perf_mode=DR,
                        )
                    return mm

                chain(0, True).then_inc(s_p, 1)   # s_p = 1
                chain(1, False).then_inc(s_p, 1)  # s_p = 2
                # m0 transposes (fp32): bank k%4; banks 0/1 reused at k=4,5
                pe.wait_ge(s_g, 2)
                TRM0_FREED = [1, 1, 1, 1, 2, 2]
                TRM0_BANKS = [ps_t[0], ps_t[1], ps_t[2], ps_t[3], ps_s[0], ps_s[2]]
                for k in range(6):
                    pe.wait_ge(s_a, TRM0_FREED[k])
                    nc.tensor.matmul(
                        TRM0_BANKS[k][0:128, 0:128],
                        e_hi[:, k * 128 : (k + 1) * 128],
                        iden[0:128, 0:128],
                        is_transpose=True,
                        start=True,
                        stop=True,
                    ).then_inc(s_p, 1)               # s_p = 3..8
                # m1 score chains (q rows 128:160): 3-way col tiling, each
                # tile owns 256 positions; plain fp8 (no DR -- col tiling and
                # DR are mutually exclusive)
                for s in range(40):
                    i_, jp = s % 10, (s // 10) % 2 * 2
                    j_ = jp + (0 if s < 20 else 1)
                    ij = i_ * KW + j_
                    st, sp_ = s == 0 or s == 1, s == 38 or s == 39
                    for c in range(3):
                        mm = nc.tensor.matmul(
                            ps_s[1][32 * c : 32 * c + 32, 0:256],
                            q_sb[:, ij, 128:160],
                            z_sb[:, j_, i_ * W + 256 * c : i_ * W + 256 * c + 256],
                            start=(s == 0),
                            stop=(s == 39),
                            tile_position=(0, 32 * c),
                        )
                mm.then_inc(s_p, 1)  # s_p = 9
                TRM1_A = [9, 9, 9, 9, 0, 0]
                TRM1_D = [0, 0, 0, 0, 0, 0]
                TRM1_BANKS = [ps_s[3], ps_s[0], ps_s[2], ps_s[1], ps_t[0], ps_t[1]]
                for k in range(6):
                    if TRM1_A[k]:
                        pe.wait_ge(s_a, TRM1_A[k])
                    if TRM1_D[k]:
                        pe.wait_ge(s_d, TRM1_D[k])
                    pb = 32 * (k // 2)
                    cb = 128 * (k % 2)
                    nc.tensor.matmul(
                        TRM1_BANKS[k][0:128, 0:32],
                        e_lo[pb : pb + 32, cb : cb + 128],
                        iden[pb : pb + 32, pb : pb + 32],
                        is_transpose=True,
                        start=True,
                        stop=True,
                    ).then_inc(s_p, 1)  # s_p = 10..15
                # phase 2 m0 (DR over k-chunk pairs)
                pe.wait_ge(s_a, 8)
                PH2_BANK_FREE = [(1, 12), (2, 3), (0, 0), (0, 0),
                                 (1, 13), (2, 4), (1, 14), (2, 5),
                                 (1, 15), (2, 6)]
                for gidx in range(NT):
                    if gidx == 0:
                        pe.wait_ge(s_kv1, 16)
                    elif gidx == 3:
                        pe.wait_ge(s_kv2, 16)
                    elif gidx == 7:
                        pe.wait_ge(s_kv3, 16)
                    kind, val = PH2_BANK_FREE[gidx]
                    if kind == 1:
                        pe.wait_ge(s_a, val)
                    elif kind == 2:
                        pe.wait_ge(s_d, val)
                    for tp in range(T // 2):
                        t = 2 * tp
                        mm = nc.tensor.matmul(
                            ps_t[gidx % 4][0:128, 0:512],
                            eT_sb[:, t : t + 2, 0:128],
                            kv_sb[:, t : t + 2, gidx * 512 : (gidx + 1) * 512],
                            start=(tp == 0),
                            stop=(tp == T // 2 - 1),
                            perf_mode=DR,
                        )
                    mm.then_inc(s_p, 1)  # s_p = 16+gidx
                # m1 transposes from e_lo [96, 256]: chunk t lives at
                # partitions 32*(t//2), cols 128*(t%2)
                # phase 2 m1: rounds of col-tiled n-tiles sharing one psum
                # bank at partition bases 0/32/64/96; the small (2-tile)
                # round g8-9 runs FIRST so the last round's copy+DMA tail is
                # a full-width transfer
                pe.wait_ge(s_a, 12)
                pe.wait_ge(s_d, 3)
                RBANKS = [ps_s[0], ps_t[2], ps_t[3]]
                RWAIT = [(0, 0), (1, 16), (2, 7)]
                RG = [(8, 2), (0, 4), (4, 4)]
                for r in range(3):
                    g0, ntile = RG[r]
                    bank = RBANKS[r]
                    kind, val = RWAIT[r]
                    if kind == 1:
                        pe.wait_ge(s_a, val)
                    elif kind == 2:
                        pe.wait_ge(s_d, val)
                    for t in range(T):
                        for c in range(ntile):
                            gcol = (g0 + c) * 512
                            mm = nc.tensor.matmul(
                                bank[32 * c : 32 * c + 32, 0:512],
                                eT_sb[:, t, 128:160],
                                kv_sb[:, t, gcol : gcol + 512],
                                start=(t == 0),
                                stop=(t == T - 1),
                                tile_position=(0, 32 * c),
                            )
                    mm.then_inc(s_p, 1)  # s_p = 26+r

            @block.scalar
            def _(act):
                # preload the EXP activation table while idle
                act.wait_ge(s_g, 3)
                nc.scalar.activation(
                    e_hi[0:1, 0:1], bias0[0:1, 0:1], AF.Exp, bias=0.0, scale=1.0
                )
                # build z slab 3 on-chip (slab0 shifted by 3 bytes)
                act.wait_ge(s_z1, 16)
                nc.scalar.activation(
                    z_sb[:, 3, 0 : ZFL - 3], z_sb[:, 0, 3:ZFL], AF.Copy
                ).then_inc(s_z2, 1)

                def expcall(ps, esl, b):
                    # e' = 64 * exp(s): bias ln(64) keeps the centered
                    # f' = e' - 64 in e4m3's well-resolved range.
                    nc.scalar.activation(
                        esl, ps, AF.Exp, bias=b, scale=SCALE
                    ).then_inc(s_a, 1)

                act.wait_ge(s_p, 1)
                expcall(ps_s[0][:, :], e_hi[:, 0:N0], bias0[:, :])        # s_a=1
                act.wait_ge(s_p, 2)
                expcall(ps_s[2][:, :], e_hi[:, OFF1 : OFF1 + N1], bias0[:, :])  # 2
                TRC0_BANKS = [ps_t[0], ps_t[1], ps_t[2], ps_t[3], ps_s[0], ps_s[2]]
                for k in range(6):
                    # transposed m0 chunks -> f' = e' - 64 (cast to e4m3)
                    act.wait_ge(s_p, 3 + k)
                    nc.scalar.activation(
                        eT_sb[:, k, 0:128],
                        TRC0_BANKS[k][0:128, 0:128],
                        AF.Copy,
                        bias=-64.0,
                    ).then_inc(s_a, 1)                                   # s_a=3..8
                act.wait_ge(s_p, 9)
                expcall(ps_s[1][0:96, 0:256], e_lo[0:96, 0:256], bias0[0:96, :])  # 9
                for k, tbank in ((0, ps_s[3]), (2, ps_s[2]), (4, ps_t[0])):
                    act.wait_ge(s_p, 10 + k)
                    nc.scalar.activation(
                        eT_sb[:, k, 128:160],
                        tbank[0:128, 0:32],
                        AF.Copy,
                        bias=-64.0,
                    ).then_inc(s_a, 1)  # s_a = 10, 11, 12
                for h in range(5):
                    # ph2-m0 even-g out copies (odd g on the DVE)
                    g2 = 2 * h
                    act.wait_ge(s_p, 16 + g2)
                    nc.scalar.activation(
                        o_hi[:, g2 * 512 : (g2 + 1) * 512],
                        ps_t[g2 % 4][0:128, 0:512],
                        AF.Copy,
                    ).then_inc(s_a, 1)  # s_a = 13+h
                act.wait_ge(s_p, 26)
                nc.scalar.activation(
                    o_lo[0:64, 0, :], ps_s[0][0:64, 0:512], AF.Copy
                ).then_inc(s_a, 1)  # s_a = 18: round 0 (g8-9)
                act.wait_ge(s_p, 28)
                nc.scalar.activation(
                    o_lo[:, 2, :], ps_t[3][0:128, 0:512], AF.Copy
                ).then_inc(s_a, 1)  # s_a = 19: round 2 (g4-7)

            @block.vector
            def _(dve):
                # build z slab 2 on-chip (slab0 shifted by 2 bytes)
                dve.wait_ge(s_z1, 16)
                nc.vector.tensor_copy(
                    z_sb[:, 2, 0 : ZFL - 2], z_sb[:, 0, 2:ZFL]
                ).then_inc(s_z2, 1)
                dve.wait_ge(s_a, 2)
                nc.vector.reduce_sum(
                    dh_sb[:], e_hi[:, :], axis=mybir.AxisListType.X
                ).then_inc(s_v, 1)
                # ph2-m0 odd-g out copies
                for k, tbank in ((1, ps_s[0]), (3, ps_s[1]), (5, ps_t[1])):
                    dve.wait_ge(s_p, 10 + k)
                    nc.vector.tensor_scalar_add(
                        eT_sb[:, k, 128:160],
                        tbank[0:128, 0:32],
                        -64.0,
                    ).then_inc(s_d, 1)  # s_d = 1, 2, 3
                dve.wait_ge(s_a, 9)
                for c in range(3):
                    nc.vector.reduce_sum(
                        dl_sb[32 * c : 32 * c + 32, :],
                        e_lo[32 * c : 32 * c + 32, :],
                        axis=mybir.AxisListType.X,
                    ).then_inc(s_v, 1)  # s_v = 2..4
                for h in range(5):
                    g2 = 2 * h + 1
                    dve.wait_ge(s_p, 16 + g2)
                    nc.vector.tensor_copy(
                        o_hi[:, g2 * 512 : (g2 + 1) * 512],
                        ps_t[g2 % 4][0:128, 0:512],
                    ).then_inc(s_d, 1)  # s_d = 4+h
                dve.wait_ge(s_p, 27)
                nc.vector.tensor_copy(
                    o_lo[:, 1, :], ps_t[2][0:128, 0:512]
                ).then_inc(s_d, 1)  # s_d = 9: ph2-m1 round 1 (g0-3)

    return nc


def _host_prep(z1_hat, z2):
    z1 = np.asarray(z1_hat, dtype=np.float32)[0]  # [128, 100, 64]
    z2a = np.asarray(z2, dtype=np.float32)[0]

    # q patches [160, 5120] and lhsT layout qT3 [128, 40, 160]
    q = z1.reshape(KC, NH, KH, NW, KW).transpose(1, 3, 0, 2, 4).reshape(PQ, D)
    qT3 = np.ascontiguousarray(q.reshape(PQ, KC, KH * KW).transpose(1, 2, 0).astype(F8))

    # padded z2: rows 100..111 zero
    z_pad = np.zeros((KC, 112, W), dtype=np.float32)
    z_pad[:, :H] = z2a

    # sliding kv patches from padded z2
    sw = np.lib.stride_tricks.sliding_window_view(z_pad, (KH, KW), axis=(1, 2))
    # sw: [128, 103, 61, 10, 4]; patch(h, w) = sw[:, h, w]

    q64 = q.astype(np.float64)
    ij_off = (np.arange(KH)[:, None] * W + np.arange(KW)[None, :]).reshape(-1)  # [40]

    in_maps = []
    corrs = []
    for core in range(NCORES):
        h0 = HPC * core
        zf = z_pad[:, h0 : h0 + ZROWS, :].reshape(KC, ZFL)
        # the 4 byte-shifted slabs, quantized once so all shifts agree
        zf8 = np.zeros((KC, ZFL + KW), dtype=F8)
        zf8[:, :ZFL] = zf.astype(F8)
        z4 = np.stack([zf8[:, s : s + ZFL] for s in range(KW)], axis=1)
        # kv rows indexed by flat position p = h_local*64 + w
        kvp = np.zeros((PKC, D), dtype=np.float32)
        hh = np.arange(PKC) // W
        ww = np.arange(PKC) % W
        real = (ww < WK) & (h0 + hh < HK)
        ridx = np.nonzero(real)[0]
        kvp[ridx] = (
            sw[:, h0 + hh[ridx], ww[ridx]].transpose(1, 0, 2, 3).reshape(-1, D)
        )
        kvr = np.ascontiguousarray(
            kvp.reshape(T, 128, D).transpose(1, 0, 2).astype(F8)
        )
        in_maps.append(
            {
                "z4": np.ascontiguousarray(z4),
                "qT3": qT3,
                "kvr": kvr,
            }
        )
        # denominator correction: computed-but-invalid columns (w >= 61 or
        # h >= 91) polluted the on-chip row-sum; subtract their exactly
        # recomputed exp contribution.
        bad = np.nonzero(~real)[0]
        win = zf.astype(np.float64)[:, bad[:, None] + ij_off[None, :]]  # [128,nb,40]
        patches = win.transpose(1, 0, 2).reshape(len(bad), D)  # d-order (c, i, j)
        s_bad = q64 @ patches.T  # [160, nb]
        corrs.append(np.exp(s_bad * SCALE).sum(axis=1))

    corr = np.sum(corrs, axis=0)
    # centered softmax: device returns f' @ kv with f' = 64*(e-1); host adds
    # the exact colsum term sum_k kv[k, :] over all real patches (all cores).
    swr = sw[:, :HK, :WK]
    colsum = swr.astype(np.float64).sum(axis=(1, 2)).reshape(D)  # [5120]
    return in_maps, corr, colsum


def kernel(z1_hat, z2):
    from concourse.bass_utils import run_bass_kernel_spmd

    in_maps, corr, colsum = _host_prep(z1_hat, z2)
    if "nc" not in _CACHE:
        _CACHE["nc"] = _build_nc()
    nc = _CACHE["nc"]
    res = run_bass_kernel_spmd(nc, in_maps, list(range(NCORES)))
    num = np.broadcast_to(colsum, (PQ, D)).astype(np.float64).copy()
    den = -corr
    for r in res.results:
        num[0:128] += r["out"].astype(np.float64) / 64.0
        o2 = r["out2"].astype(np.float64).reshape(4, 32, 3, 512)
        m1 = np.concatenate(
            [
                o2[:, :, 1, :].transpose(1, 0, 2).reshape(32, 2048),
                o2[:, :, 2, :].transpose(1, 0, 2).reshape(32, 2048),
                o2[0:2, :, 0, :].transpose(1, 0, 2).reshape(32, 1024),
            ],
            axis=1,
        )
        num[128:160] += m1 / 64.0
        dv = r["den"].astype(np.float64)[:, 0] / 64.0
        den = den + np.concatenate(
            [dv[0:128], dv[128:160] + dv[160:192] + dv[192:224]]
        )
    out = (num / den[:, None]).astype(np.float32)
    # fold patches back: [160, 5120] -> [1, 128, 100, 64]
    out = out.reshape(NH, NW, KC, KH, KW).transpose(2, 0, 3, 1, 4)
    return np.ascontiguousarray(out.reshape(1, KC, H, W))
